# revision 16
# baseline (speedup 1.0000x reference)
"""Trainium2 Bass kernel for the GNN edge-update MLP (8 NeuronCores).

Reference semantics:
    h   = x @ W_lin.T + b_lin                       # [N, nin]
    agg = h[src] + h[dst]                           # [E, nin]
    z   = concat([agg, edge_attr], -1)              # [E, 2*nin]
    z   = relu(BN(z @ W1.T + b1; g1, be1))          # [E, nout]  (BN over edges)
    z   = relu(BN(z @ W2.T + b2; g2, be2))          # [E, nout]

Restructuring (v2 — one-hot src path):
  * b1/b2 cancel inside training-mode BN -> dropped.
  * z @ W1.T = hW[src] + hW[dst] + ea @ W1b.T, with W1 = [W1a | W1b] and
    hW = x @ (W1a W_lin).T + W1a b_lin  (a [N, nout] row-major DRAM table).
  * Edges are sharded over 8 cores; per core they are bucketed by
    (dst >= SPLIT) [int16 gather trick] and SORTED BY SRC inside each bucket.
  * dst contribution: GPSIMD dma_gather from the table (per-edge descriptors,
    ~8ns each — this is the only per-edge SWDGE cost left).
  * src contribution: because edges are src-sorted, each 512-column chunk's
    srcs span only ~2-4 aligned 128-node windows.  For each window the PE
    multiplies the table slice (lhsT [128 nodes, 128 feat]) by a one-hot
    selection matrix S (built on DVE by comparing a broadcast src-value row
    against an iota column) and accumulates into the chunk's PSUM bank.
    No per-edge descriptors at all.
  * The per-chunk window structure (and bucket caps) are data-dependent and
    baked into the compiled graph (compile happens per input set; cached).
  * BN statistics: per-chunk vector bn_stats, merged, AllReduce'd ([128,2]).
    A dummy AllReduce at t=0 absorbs the first-collective setup cost.
  * Padded columns are exactly zero through u1 (src matches no window row,
    dst gathers a zero row, ea is zero); their constant effect on the second
    BN is subtracted analytically (v2 correction), as in v1.
"""

import sys
from contextlib import ExitStack

import numpy as np

try:
    import concourse  # noqa: F401
except ImportError:  # pragma: no cover
    sys.path.insert(0, "/opt/trn_rl_repo")

import ml_dtypes
from concourse import bass, bacc, mybir
from concourse import tile
from concourse.bass_utils import run_bass_kernel_spmd
from concourse.masks import make_identity

BF16 = ml_dtypes.bfloat16
FP16 = np.float16

N_CORES = 8
NIN = 128
EPS = 1e-5
P = 128

SPLIT = 32640            # nodes < SPLIT are "lo", >= SPLIT are "hi" (128-mult)
GROUP = 2048             # edges per dma_gather instruction
CHUNK = 512
RUN = 512                # table rows per window-run load (4 windows)
MAXW = 16                # max windows (of 128 rows) spanned by one chunk


def _r512(v):
    return ((v + 511) // 512) * 512


def table_layout(n_nodes):
    """One DRAM table, hi region first (rows [0, hi_rows)), then lo."""
    nhi = n_nodes - SPLIT
    hi_rows = _r512(nhi + 1)
    lo_rows = _r512(SPLIT + 1)
    return nhi, hi_rows, lo_rows, hi_rows + lo_rows


class Layout:
    """Per-compile structural data (hashable via .key)."""

    def __init__(self, caps, chunks, groups, runs, n_nodes, e_total):
        self.caps = caps          # (cap_hi, cap_lo) in BUCKET order (hi, lo)
        self.chunks = chunks      # [(off, gi, [(run_i, k, dw), ...])]
        self.groups = groups      # [(off, L, dst_hi)]
        self.runs = runs          # [table_row0] per run (RUN rows each)
        self.n_nodes = n_nodes
        self.e_total = e_total
        self.ec = sum(caps)
        self.key = (caps, tuple((o, g, tuple(w)) for o, g, w in chunks),
                    tuple(groups), tuple(runs), n_nodes, e_total)


def build_graph(lay: Layout, n_cores=N_CORES, eps=EPS):
    f32 = mybir.dt.float32
    bf16 = mybir.dt.bfloat16
    fp16 = mybir.dt.float16
    f8 = mybir.dt.float8e4
    i16 = mybir.dt.int16
    FT = mybir.ActivationFunctionType

    nc = bacc.Bacc(
        "TRN2", target_bir_lowering=False, debug=False, num_devices=n_cores
    )

    nhi, hi_rows, lo_rows, npad = table_layout(lay.n_nodes)
    ec = lay.ec
    e_total = lay.e_total
    n_pad_tot = ec * n_cores - e_total
    nchunk = ec // CHUNK
    nstat = nchunk

    # ---- I/O -------------------------------------------------------------
    eaT = nc.dram_tensor("eaT", [P, ec], f8, kind="ExternalInput").ap()
    xT = nc.dram_tensor("xT", [P, npad], bf16, kind="ExternalInput").ap()
    srel = nc.dram_tensor("srel", [1, ec], fp16, kind="ExternalInput").ap()
    didx = nc.dram_tensor("didx", [P, ec // 16], i16, kind="ExternalInput").ap()
    iota = nc.dram_tensor("iota", [P, MAXW], f32, kind="ExternalInput").ap()
    wlin = nc.dram_tensor("wlin", [P, P], f32, kind="ExternalInput").ap()
    w1 = nc.dram_tensor("w1", [P, 2 * P], f32, kind="ExternalInput").ap()
    w2 = nc.dram_tensor("w2", [P, P], f32, kind="ExternalInput").ap()
    blin = nc.dram_tensor("blin", [P, 1], f32, kind="ExternalInput").ap()
    g1 = nc.dram_tensor("g1", [P, 1], f32, kind="ExternalInput").ap()
    be1 = nc.dram_tensor("be1", [P, 1], f32, kind="ExternalInput").ap()
    g2 = nc.dram_tensor("g2", [P, 1], f32, kind="ExternalInput").ap()
    be2 = nc.dram_tensor("be2", [P, 1], f32, kind="ExternalInput").ap()
    outT = nc.dram_tensor("outT", [P, ec], bf16, kind="ExternalOutput").ap()

    table = nc.dram_tensor("hw_table", [npad, P], bf16).ap()

    grp_all = [list(range(n_cores))]

    with tile.TileContext(nc) as tc, ExitStack() as es:
        consts = es.enter_context(tc.tile_pool(name="consts", bufs=1))
        gidx = es.enter_context(tc.tile_pool(name="gidx", bufs=4))
        dram = es.enter_context(tc.tile_pool(name="dram", bufs=1, space="DRAM"))
        big = es.enter_context(tc.tile_pool(name="big", bufs=1))
        red = es.enter_context(tc.tile_pool(name="red", bufs=1))

        # ---- warm-up collective (absorbs first-cc setup latency) --------
        wu_in = dram.tile([P, 2], f32, tag="wu_in")
        wu_sb = red.tile([P, 2], f32, tag="wu_sb")
        nc.vector.memset(wu_sb[:], 0.0)
        nc.sync.dma_start(out=wu_in[:], in_=wu_sb[:])
        wu_out = dram.tile([P, 2], f32, tag="wu_out")
        nc.gpsimd.collective_compute(
            "AllReduce", mybir.AluOpType.add, replica_groups=grp_all,
            ins=[wu_in[:].opt()], outs=[wu_out[:].opt()])

        # ---- constants / weight prep ------------------------------------
        ident_f = consts.tile([P, P], f32)
        make_identity(nc, ident_f[:])

        wlin_s = consts.tile([P, P], f32)
        nc.sync.dma_start(out=wlin_s[:], in_=wlin)
        w1_s = consts.tile([P, 2 * P], f32)
        nc.sync.dma_start(out=w1_s[:], in_=w1)
        w2_s = consts.tile([P, P], f32)
        nc.sync.dma_start(out=w2_s[:], in_=w2)
        blin_s = consts.tile([P, 1], f32)
        nc.sync.dma_start(out=blin_s[:], in_=blin)
        g1_s = consts.tile([P, 1], f32)
        nc.sync.dma_start(out=g1_s[:], in_=g1)
        be1_s = consts.tile([P, 1], f32)
        nc.sync.dma_start(out=be1_s[:], in_=be1)
        g2_s = consts.tile([P, 1], f32)
        nc.sync.dma_start(out=g2_s[:], in_=g2)
        be2_s = consts.tile([P, 1], f32)
        nc.sync.dma_start(out=be2_s[:], in_=be2)
        iota_s = consts.tile([P, MAXW], f32)
        nc.sync.dma_start(out=iota_s[:], in_=iota)
        eps_s = consts.tile([P, 1], f32)
        nc.vector.memset(eps_s[:], eps)
        ones16 = consts.tile([1, P], fp16)
        nc.vector.memset(ones16[:], 1.0)

        # prefetch dst idx for the first groups while the table builds
        idx_pre = {}
        for gi, (off, L, _dh) in enumerate(lay.groups[:4]):
            di = gidx.tile([P, GROUP // 16], i16, tag="di")
            nc.sync.dma_start(out=di[:, :L // 16],
                              in_=didx[:, off // 16:(off + L) // 16])
            idx_pre[gi] = di

        w1aT = consts.tile([P, P], f32)
        w1bT = consts.tile([P, P], bf16)
        w2T = consts.tile([P, P], bf16)
        wcT = consts.tile([P, P], bf16)
        bc = consts.tile([P, 1], f32)

        with tc.tile_pool(name="psum0", bufs=1, space="PSUM") as psw, \
             tc.tile_pool(name="psum0b", bufs=3, space="PSUM") as ps0:
            pw = psw.tile([P, P], f32, tag="pw")
            nc.tensor.matmul(pw[:], lhsT=w1_s[:, 0:P], rhs=ident_f[:],
                             start=True, stop=True)
            nc.vector.tensor_copy(w1aT[:], pw[:])
            pw = psw.tile([P, P], f32, tag="pw")
            nc.tensor.matmul(pw[:], lhsT=w1_s[:, P:2 * P], rhs=ident_f[:],
                             start=True, stop=True)
            nc.vector.tensor_copy(w1bT[:], pw[:])
            pw = psw.tile([P, P], f32, tag="pw")
            nc.tensor.matmul(pw[:], lhsT=w2_s[:], rhs=ident_f[:],
                             start=True, stop=True)
            nc.vector.tensor_copy(w2T[:], pw[:])
            # WcT[i, o] = (W1a @ W_lin)[o, i]
            pw = psw.tile([P, P], f32, tag="pw")
            nc.tensor.matmul(pw[:], lhsT=wlin_s[:], rhs=w1aT[:],
                             start=True, stop=True)
            nc.vector.tensor_copy(wcT[:], pw[:])
            pb = psw.tile([P, 1], f32, tag="pb")
            nc.tensor.matmul(pb[:], lhsT=w1aT[:], rhs=blin_s[:],
                             start=True, stop=True)
            nc.vector.tensor_copy(bc[:], pb[:])

            ident_b = consts.tile([P, P], bf16)
            nc.vector.tensor_copy(ident_b[:], ident_f[:])

            # ---- phase 0: build the hW table (hi region first) ----------
            with tc.tile_pool(name="ph0", bufs=4) as ph0:
                zrow = ph0.tile([P, P], bf16, tag="zrow")
                nc.vector.memset(zrow[:], 0.0)

                def build(row0, xcol0, nch):
                    for j in range(nch):
                        xt = ph0.tile([P, 512], bf16, tag="xt")
                        nc.sync.dma_start(
                            out=xt[:],
                            in_=xT[:, xcol0 + j * 512:xcol0 + (j + 1) * 512])
                        hp = ps0.tile([P, 512], f32, tag="hp")
                        nc.tensor.matmul(hp[:], lhsT=wcT[:], rhs=xt[:],
                                         start=True, stop=True)
                        hs = ph0.tile([P, 512], bf16, tag="hs")
                        nc.scalar.activation(hs[:], hp[:], func=FT.Identity,
                                             bias=bc[:], scale=1.0)
                        tp = ps0.tile([P, 512], f32, tag="tp")
                        for s in range(4):
                            nc.tensor.matmul(tp[:, s * P:(s + 1) * P],
                                             lhsT=hs[:, s * P:(s + 1) * P],
                                             rhs=ident_b[:], start=True,
                                             stop=True)
                        ts = ph0.tile([P, 512], bf16, tag="ts")
                        nc.vector.tensor_copy(ts[:], tp[:])
                        r = row0 + j * 512
                        nc.sync.dma_start(
                            out=table[r:r + 512, :].rearrange(
                                "(s p) o -> p s o", p=P),
                            in_=ts[:].rearrange("p (s o) -> p s o", s=4),
                        )

                build(0, 0, hi_rows // 512)
                nc.sync.dma_start(out=table[nhi:nhi + 1, :], in_=zrow[0:1, :])
                build(hi_rows, hi_rows, lo_rows // 512)
                zlo = hi_rows + SPLIT
                nc.sync.dma_start(out=table[zlo:zlo + 1, :], in_=zrow[0:1, :])

        u1 = big.tile([P, ec], bf16)
        stats = consts.tile([P, nstat, 6], f32)

        def bn_coeffs(g_s, be_s, nck, corr=None):
            se = red.tile([P, nstat], f32, tag="se")
            nc.vector.tensor_mul(se[:, :nck], stats[:, :nck, 0],
                                 stats[:, :nck, 1])
            so = red.tile([P, nstat], f32, tag="so")
            nc.vector.tensor_mul(so[:, :nck], stats[:, :nck, 3],
                                 stats[:, :nck, 4])
            qe = red.tile([P, nstat], f32, tag="qe")
            nc.vector.tensor_mul(qe[:, :nck], se[:, :nck], stats[:, :nck, 1])
            nc.vector.tensor_add(qe[:, :nck], qe[:, :nck], stats[:, :nck, 2])
            qo = red.tile([P, nstat], f32, tag="qo")
            nc.vector.tensor_mul(qo[:, :nck], so[:, :nck], stats[:, :nck, 4])
            nc.vector.tensor_add(qo[:, :nck], qo[:, :nck], stats[:, :nck, 5])
            nc.vector.tensor_add(se[:, :nck], se[:, :nck], so[:, :nck])
            nc.vector.tensor_add(qe[:, :nck], qe[:, :nck], qo[:, :nck])
            sq = red.tile([P, 2], f32, tag="sq")
            nc.vector.tensor_reduce(sq[:, 0:1], se[:, :nck],
                                    axis=mybir.AxisListType.X,
                                    op=mybir.AluOpType.add)
            nc.vector.tensor_reduce(sq[:, 1:2], qe[:, :nck],
                                    axis=mybir.AxisListType.X,
                                    op=mybir.AluOpType.add)
            cc_in = dram.tile([P, 2], f32, tag="cc_in")
            nc.sync.dma_start(out=cc_in[:], in_=sq[:])
            cc_out = dram.tile([P, 2], f32, tag="cc_out")
            nc.gpsimd.collective_compute(
                "AllReduce", mybir.AluOpType.add, replica_groups=grp_all,
                ins=[cc_in[:].opt()], outs=[cc_out[:].opt()])
            sqg = red.tile([P, 2], f32, tag="sqg")
            nc.sync.dma_start(out=sqg[:], in_=cc_out[:])
            if corr is not None:
                v, vq = corr
                t = red.tile([P, 2], f32, tag="tcorr")
                nc.vector.tensor_scalar_mul(t[:, 0:1], v[:], float(n_pad_tot))
                nc.vector.tensor_scalar_mul(t[:, 1:2], vq[:], float(n_pad_tot))
                nc.vector.tensor_sub(sqg[:], sqg[:], t[:])
            mu = red.tile([P, 1], f32, tag="mu")
            nc.vector.tensor_scalar_mul(mu[:], sqg[:, 0:1], 1.0 / e_total)
            var = red.tile([P, 1], f32, tag="var")
            nc.vector.tensor_scalar_mul(var[:], sqg[:, 1:2], 1.0 / e_total)
            mu2 = red.tile([P, 1], f32, tag="mu2")
            nc.vector.tensor_mul(mu2[:], mu[:], mu[:])
            nc.vector.tensor_sub(var[:], var[:], mu2[:])
            a = red.tile([P, 1], f32, tag="a")
            nc.scalar.activation(a[:], var[:], func=FT.Sqrt, bias=eps_s[:],
                                 scale=1.0)
            nc.vector.reciprocal(a[:], a[:])
            nc.vector.tensor_mul(a[:], a[:], g_s[:])
            c = red.tile([P, 1], f32, tag="c")
            nc.vector.tensor_mul(c[:], mu[:], a[:])
            nc.vector.tensor_sub(c[:], be_s[:], c[:])
            return a, c

        with (
            tc.tile_pool(name="psA", bufs=4, space="PSUM") as psA,
            tc.tile_pool(name="psB", bufs=2, space="PSUM") as psB,
            tc.tile_pool(name="psS", bufs=1, space="PSUM") as psS,
            tc.tile_pool(name="ea", bufs=2) as eap,
            tc.tile_pool(name="sr", bufs=3) as srp,
            tc.tile_pool(name="gp", bufs=3) as gp,
            tc.tile_pool(name="wr", bufs=3) as wrp,
            tc.tile_pool(name="sp", bufs=3) as Sp,
            tc.tile_pool(name="op", bufs=2) as op,
        ):
            # ---- pass A ------------------------------------------------
            # dst gathers, one per group (queued up front; gp bufs throttle)
            g_tiles = {}
            for gi, (off, L, dst_hi) in enumerate(lay.groups):
                if gi in idx_pre:
                    di = idx_pre[gi]
                else:
                    di = gidx.tile([P, GROUP // 16], i16, tag="di")
                    nc.sync.dma_start(out=di[:, :L // 16],
                                      in_=didx[:, off // 16:(off + L) // 16])
                gdst = gp.tile([P, GROUP], bf16, tag="gdst")
                base = table[0:hi_rows, :] if dst_hi \
                    else table[hi_rows:npad, :]
                nc.gpsimd.dma_gather(
                    out_ap=gdst[:, :L].rearrange("p (a s) -> p a s", a=1),
                    in_ap=base, idxs_ap=di[:, :L // 16],
                    num_idxs=L, num_idxs_reg=L, elem_size=P,
                    transpose=True, single_packet=False)
                g_tiles[gi] = (gdst, off)

            # group-level ea staging
            ea_tiles = {}

            run_tiles = {}
            caps_hi = lay.caps[0]

            for ci, (off, gi, wins) in enumerate(lay.chunks):
                bkt = 0 if off < caps_hi else 1
                goff, gL, _ = lay.groups[gi]
                if gi not in ea_tiles:
                    et = eap.tile([P, GROUP], f8, tag="ea")
                    nc.sync.dma_start(out=et[:, :gL],
                                      in_=eaT[:, goff:goff + gL])
                    ea_tiles = {gi: et}
                et = ea_tiles[gi]
                rel = off - goff

                up = psA.tile([P, CHUNK], f32, tag="up")
                nc.tensor.matmul(up[:], lhsT=w1bT[:], rhs=et[:, rel:rel + CHUNK],
                                 start=True, stop=(len(wins) == 0),
                                 skip_group_check=bool(wins))

                if wins:
                    st = srp.tile([1, CHUNK], fp16, tag="sr")
                    nc.scalar.dma_start(out=st[:],
                                        in_=srel[:, off:off + CHUNK])
                    bps = psB.tile([P, CHUNK], f32, tag="bps")
                    nc.tensor.matmul(bps[:], lhsT=ones16[:], rhs=st[:],
                                     start=True, stop=True)
                    for wi, (run_i, k, dw, a, b) in enumerate(wins):
                        rkey = (bkt, run_i)
                        if rkey not in run_tiles:
                            wt = wrp.tile([P, RUN], bf16, tag="wt")
                            r0 = lay.runs[run_i]
                            nc.scalar.dma_start(
                                out=wt[:].rearrange("p (k f) -> p k f",
                                                    k=RUN // P),
                                in_=table[r0:r0 + RUN, :].rearrange(
                                    "(k p) f -> p k f", p=P))
                            run_tiles[rkey] = wt
                        wt = run_tiles[rkey]
                        S_w = Sp.tile([P, CHUNK], bf16, tag="S")
                        nc.vector.tensor_scalar(
                            out=S_w[:, a:b], in0=bps[:, a:b],
                            scalar1=iota_s[:, dw:dw + 1], scalar2=None,
                            op0=mybir.AluOpType.is_equal)
                        nc.tensor.matmul(up[:, a:b],
                                         lhsT=wt[:, k * P:(k + 1) * P],
                                         rhs=S_w[:, a:b], start=False,
                                         stop=(wi == len(wins) - 1),
                                         skip_group_check=True)

                gdst, g_off = g_tiles[gi]
                grel = off - g_off
                nc.vector.tensor_add(u1[:, off:off + CHUNK], up[:],
                                     gdst[:, grel:grel + CHUNK])
                nc.vector.bn_stats(stats[:, ci, :], u1[:, off:off + CHUNK])

            a1, c1 = bn_coeffs(g1_s, be1_s, nchunk)

            # pad columns have u1 == 0 -> u2_pad = W2 @ relu(c1), constant
            rc = red.tile([P, 1], f32, tag="rc")
            nc.scalar.activation(rc[:], c1[:], func=FT.Relu)
            rcb = red.tile([P, 1], bf16, tag="rcb")
            nc.vector.tensor_copy(rcb[:], rc[:])
            vp = psS.tile([P, 1], f32, tag="vp")
            nc.tensor.matmul(vp[:], lhsT=w2T[:], rhs=rcb[:],
                             start=True, stop=True)
            v2 = red.tile([P, 1], f32, tag="v2")
            nc.vector.tensor_copy(v2[:], vp[:])
            v2q = red.tile([P, 1], f32, tag="v2q")
            nc.vector.tensor_mul(v2q[:], v2[:], v2[:])

            # ---- pass B: z1 = relu(a1*u1+c1) in place; stats of W2@z1 ---
            for k in range(nchunk):
                off = k * CHUNK
                nc.scalar.activation(u1[:, off:off + CHUNK],
                                     u1[:, off:off + CHUNK],
                                     func=FT.Relu, scale=a1[:], bias=c1[:])
                up = psA.tile([P, CHUNK], f32, tag="up")
                nc.tensor.matmul(up[:], lhsT=w2T[:],
                                 rhs=u1[:, off:off + CHUNK],
                                 start=True, stop=True)
                nc.vector.bn_stats(stats[:, k, :], up[:])

            a2, c2 = bn_coeffs(g2_s, be2_s, nchunk, corr=(v2, v2q))

            # ---- pass C: out = relu(a2*(W2@z1)+c2), staged per group ----
            for base in range(0, ec, GROUP):
                gL = min(GROUP, ec - base)
                ot = op.tile([P, GROUP], bf16, tag="ot")
                for off in range(base, base + gL, CHUNK):
                    up = psA.tile([P, CHUNK], f32, tag="up")
                    nc.tensor.matmul(up[:], lhsT=w2T[:],
                                     rhs=u1[:, off:off + CHUNK],
                                     start=True, stop=True)
                    r = off - base
                    nc.scalar.activation(ot[:, r:r + CHUNK], up[:],
                                         func=FT.Relu, scale=a2[:],
                                         bias=c2[:])
                nc.sync.dma_start(out=outT[:, base:base + gL],
                                  in_=ot[:, :gL])

    nc.compile()
    return nc


def _wrap16(a):
    """linear [L] -> [16, L/16] wrapped, tiled to [128, L/16]."""
    w = np.ascontiguousarray(a.reshape(-1, 16).T)
    return np.tile(w, (8, 1))


def host_prep(x, edge_index, edge_attr, n_cores):
    """Shard, bucket by dst-region, sort by src, build layout + per-core
    arrays."""
    n = x.shape[0]
    e = edge_attr.shape[0]
    ec0 = e // n_cores
    nhi, hi_rows, lo_rows, npad = table_layout(n)

    src_all = edge_index[0].astype(np.int64)
    dst_all = edge_index[1].astype(np.int64)

    per_core = []
    counts = np.zeros((n_cores, 2), np.int64)
    for c in range(n_cores):
        sl = slice(c * ec0, (c + 1) * ec0)
        s, d = src_all[sl], dst_all[sl]
        hi = (d >= SPLIT).astype(np.int64)
        order = np.argsort(hi * (1 << 32) + s, kind="stable")
        counts[c, 1] = int(hi.sum())          # bucket 1 = hi
        counts[c, 0] = ec0 - counts[c, 1]
        per_core.append((s, d, hi, order))

    # bucket order: hi first (table hi region builds first), caps %512
    caps = (int(_r512(counts[:, 1].max())), int(_r512(counts[:, 0].max())))
    ec = caps[0] + caps[1]
    bucket_off = {1: 0, 0: caps[0]}

    zero_hi = nhi                 # local idx of zero row in hi region
    zero_lo = SPLIT               # local idx of zero row in lo region

    # groups (shared across cores: same caps)
    groups = []
    for b, cap in ((1, caps[0]), (0, caps[1])):
        off = bucket_off[b]
        rem = cap
        while rem > 0:
            L = min(GROUP, rem)
            groups.append((off, L, b == 1))
            off += L
            rem -= L

    # per-core padded arrays + union chunk windows
    all_srel = []
    all_didx = []
    all_eacols = []
    all_inv = []
    win_ranges = [dict() for _ in range(ec // CHUNK)]
    w0_arr = np.zeros(ec // CHUNK, np.int64)

    # first pass: compute padded src arrays to derive union windows
    src_p_all = []
    for c in range(n_cores):
        s, d, hi, order = per_core[c]
        src_p = np.full(ec, -1, np.int64)
        dst_p = np.empty(ec, np.int64)
        ea_cols = np.full(ec, -1, np.int64)
        # order sorts by (hi, src): lo bucket first in order, but hi bucket
        # comes first in columns.
        n_lo = int(counts[c, 0])
        idx_lo = order[:n_lo]
        idx_hi = order[n_lo:]
        for b, idx_b in ((1, idx_hi), (0, idx_lo)):
            cnt = len(idx_b)
            off = bucket_off[b]
            pos = off + np.arange(cnt)
            src_p[pos] = s[idx_b]
            dst_p[pos] = d[idx_b] - (SPLIT if b == 1 else 0)
            ea_cols[pos] = idx_b
            padr = np.arange(off + cnt, off + (caps[0] if b == 1 else caps[1]))
            dst_p[padr] = zero_hi if b == 1 else zero_lo
        inv = np.empty(ec0, np.int64)
        inv[idx_hi] = bucket_off[1] + np.arange(len(idx_hi))
        inv[idx_lo] = bucket_off[0] + np.arange(len(idx_lo))
        src_p_all.append(src_p)
        all_didx.append(dst_p)
        all_eacols.append(ea_cols)
        all_inv.append(inv)

        for ci in range(ec // CHUNK):
            seg = src_p[ci * CHUNK:(ci + 1) * CHUNK]
            for w in np.unique(seg[seg >= 0] // P):
                pos = np.nonzero(seg // P == w)[0]
                a, b = int(pos[0]), int(pos[-1]) + 1
                cur = win_ranges[ci].get(int(w))
                if cur is None:
                    win_ranges[ci][int(w)] = [a, b]
                else:
                    cur[0] = min(cur[0], a)
                    cur[1] = max(cur[1], b)

    # per-chunk base window + srel arrays (shared w0 across cores)
    chunks = []
    run_index = {}
    runs = []
    gi_of_off = {off: i for i, (off, L, _) in enumerate(groups)
                 for off in range(off, off + L, CHUNK)}
    for ci in range(ec // CHUNK):
        wins = sorted(win_ranges[ci])
        off = ci * CHUNK
        wl = []
        if wins:
            w0 = wins[0]
            w0_arr[ci] = w0
            assert wins[-1] - w0 < MAXW, f"chunk {ci} spans {wins}"
            for w in wins:
                node0 = w * P
                if node0 >= SPLIT:
                    row0 = node0 - SPLIT          # hi region
                else:
                    row0 = hi_rows + node0        # lo region
                run0 = (row0 // RUN) * RUN
                if run0 not in run_index:
                    run_index[run0] = len(runs)
                    runs.append(run0)
                a, b = win_ranges[ci][w]
                wl.append((run_index[run0], (row0 - run0) // P, w - w0,
                           a, b))
        chunks.append((off, gi_of_off[off], wl))

    for c in range(n_cores):
        src_p = src_p_all[c]
        srel = np.full(ec, -1.0, np.float32)
        for ci in range(ec // CHUNK):
            seg = src_p[ci * CHUNK:(ci + 1) * CHUNK]
            m = seg >= 0
            srel[ci * CHUNK:(ci + 1) * CHUNK][m] = seg[m] - w0_arr[ci] * P
        assert srel.max() < 2048
        all_srel.append(srel.astype(FP16))

    lay = Layout(caps, chunks, groups, runs, n, e)
    return lay, all_srel, all_didx, all_eacols, all_inv


def make_in_maps(x, edge_index, edge_attr, W_lin, b_lin, W1, g1, be1, W2,
                 g2, be2, n_cores):
    n = x.shape[0]
    nhi, hi_rows, lo_rows, npad = table_layout(n)
    lay, all_srel, all_didx, all_eacols, all_inv = host_prep(
        x, edge_index, edge_attr, n_cores)
    ec = lay.ec
    ec0 = edge_attr.shape[0] // n_cores

    xbf = x.astype(BF16)
    xT = np.zeros((P, npad), dtype=BF16)
    xT[:, 0:nhi] = xbf[SPLIT:n].T
    xT[:, hi_rows:hi_rows + SPLIT] = xbf[0:SPLIT].T

    iota = (np.arange(P)[:, None]
            + P * np.arange(MAXW)[None, :]).astype(np.float32)

    f32c = np.ascontiguousarray
    wlin_h = f32c(W_lin.astype(np.float32))
    w1_h = f32c(W1.astype(np.float32))
    w2_h = f32c(W2.astype(np.float32))
    blin_h = f32c(b_lin.astype(np.float32).reshape(P, 1))
    g1_h = f32c(g1.astype(np.float32).reshape(P, 1))
    be1_h = f32c(be1.astype(np.float32).reshape(P, 1))
    g2_h = f32c(g2.astype(np.float32).reshape(P, 1))
    be2_h = f32c(be2.astype(np.float32).reshape(P, 1))

    eabf = edge_attr.astype(ml_dtypes.float8_e4m3fn)

    in_maps = []
    for c in range(n_cores):
        ea_cols = all_eacols[c]
        eaT = np.zeros((P, ec), dtype=ml_dtypes.float8_e4m3fn)
        real = ea_cols >= 0
        eaT[:, real] = eabf[c * ec0 + ea_cols[real]].T
        dw = np.zeros((P, ec // 16), np.int16)
        for off, L, _ in lay.groups:
            dw[:, off // 16:(off + L) // 16] = _wrap16(
                all_didx[c][off:off + L].astype(np.int16))
        in_maps.append({
            "eaT": eaT, "xT": xT, "srel": all_srel[c].reshape(1, ec),
            "didx": dw, "iota": iota,
            "wlin": wlin_h, "w1": w1_h, "w2": w2_h, "blin": blin_h,
            "g1": g1_h, "be1": be1_h, "g2": g2_h, "be2": be2_h,
        })
    return lay, in_maps, all_inv


_GRAPH_CACHE = {}


def get_graph(lay: Layout):
    if lay.key not in _GRAPH_CACHE:
        _GRAPH_CACHE[lay.key] = build_graph(lay)
    return _GRAPH_CACHE[lay.key]


def kernel(x, edge_index, edge_attr, W_lin, b_lin, W1, b1, g1, be1, W2, b2,
           g2, be2):
    """Full-input entry point: shard, run on 8 NeuronCores, gather."""
    x = np.asarray(x)
    edge_index = np.asarray(edge_index)
    edge_attr = np.asarray(edge_attr)
    e = edge_attr.shape[0]
    ec0 = e // N_CORES

    lay, in_maps, invs = make_in_maps(
        x, edge_index, edge_attr, np.asarray(W_lin), np.asarray(b_lin),
        np.asarray(W1), np.asarray(g1), np.asarray(be1), np.asarray(W2),
        np.asarray(g2), np.asarray(be2), N_CORES)
    nc = get_graph(lay)
    res = run_bass_kernel_spmd(nc, in_maps, core_ids=list(range(N_CORES)))
    out = np.empty((e, NIN), dtype=np.float32)
    for c in range(N_CORES):
        oT = np.asarray(res.results[c]["outT"], dtype=np.float32)
        out[c * ec0:(c + 1) * ec0] = oT.T[invs[c]]
    return out


# revision 17
# speedup vs baseline: 1.0045x; 1.0045x over previous
"""Trainium2 Bass kernel for the GNN edge-update MLP (8 NeuronCores).

Reference semantics:
    h   = x @ W_lin.T + b_lin                       # [N, nin]
    agg = h[src] + h[dst]                           # [E, nin]
    z   = concat([agg, edge_attr], -1)              # [E, 2*nin]
    z   = relu(BN(z @ W1.T + b1; g1, be1))          # [E, nout]  (BN over edges)
    z   = relu(BN(z @ W2.T + b2; g2, be2))          # [E, nout]

Restructuring (v2 — one-hot src path):
  * b1/b2 cancel inside training-mode BN -> dropped.
  * z @ W1.T = hW[src] + hW[dst] + ea @ W1b.T, with W1 = [W1a | W1b] and
    hW = x @ (W1a W_lin).T + W1a b_lin  (a [N, nout] row-major DRAM table).
  * Edges are sharded over 8 cores; per core they are bucketed by
    (dst >= SPLIT) [int16 gather trick] and SORTED BY SRC inside each bucket.
  * dst contribution: GPSIMD dma_gather from the table (per-edge descriptors,
    ~8ns each — this is the only per-edge SWDGE cost left).
  * src contribution: because edges are src-sorted, each 512-column chunk's
    srcs span only ~2-4 aligned 128-node windows.  For each window the PE
    multiplies the table slice (lhsT [128 nodes, 128 feat]) by a one-hot
    selection matrix S (built on DVE by comparing a broadcast src-value row
    against an iota column) and accumulates into the chunk's PSUM bank.
    No per-edge descriptors at all.
  * The per-chunk window structure (and bucket caps) are data-dependent and
    baked into the compiled graph (compile happens per input set; cached).
  * BN statistics: per-chunk vector bn_stats, merged, AllReduce'd ([128,2]).
    A dummy AllReduce at t=0 absorbs the first-collective setup cost.
  * Padded columns are exactly zero through u1 (src matches no window row,
    dst gathers a zero row, ea is zero); their constant effect on the second
    BN is subtracted analytically (v2 correction), as in v1.
"""

import sys
from contextlib import ExitStack

import numpy as np

try:
    import concourse  # noqa: F401
except ImportError:  # pragma: no cover
    sys.path.insert(0, "/opt/trn_rl_repo")

import ml_dtypes
from concourse import bass, bacc, mybir
from concourse import tile
from concourse.bass_utils import run_bass_kernel_spmd
from concourse.masks import make_identity

BF16 = ml_dtypes.bfloat16
FP16 = np.float16

N_CORES = 8
NIN = 128
EPS = 1e-5
P = 128

SPLIT = 32640            # nodes < SPLIT are "lo", >= SPLIT are "hi" (128-mult)
GROUP = 2048             # edges per dma_gather instruction
CHUNK = 512
RUN = 512                # table rows per window-run load (4 windows)
MAXW = 16                # max windows (of 128 rows) spanned by one chunk


def _r512(v):
    return ((v + 511) // 512) * 512


def table_layout(n_nodes):
    """One DRAM table, hi region first (rows [0, hi_rows)), then lo."""
    nhi = n_nodes - SPLIT
    hi_rows = _r512(nhi + 1)
    lo_rows = _r512(SPLIT + 1)
    return nhi, hi_rows, lo_rows, hi_rows + lo_rows


class Layout:
    """Per-compile structural data (hashable via .key)."""

    def __init__(self, caps, chunks, groups, runs, n_nodes, e_total):
        self.caps = caps          # (cap_hi, cap_lo) in BUCKET order (hi, lo)
        self.chunks = chunks      # [(off, gi, [(run_i, k, dw), ...])]
        self.groups = groups      # [(off, L, dst_hi)]
        self.runs = runs          # [table_row0] per run (RUN rows each)
        self.n_nodes = n_nodes
        self.e_total = e_total
        self.ec = sum(caps)
        self.key = (caps, tuple((o, g, tuple(w)) for o, g, w in chunks),
                    tuple(groups), tuple(runs), n_nodes, e_total)


def build_graph(lay: Layout, n_cores=N_CORES, eps=EPS):
    f32 = mybir.dt.float32
    bf16 = mybir.dt.bfloat16
    fp16 = mybir.dt.float16
    f8 = mybir.dt.float8e4
    i16 = mybir.dt.int16
    FT = mybir.ActivationFunctionType

    nc = bacc.Bacc(
        "TRN2", target_bir_lowering=False, debug=False, num_devices=n_cores
    )

    nhi, hi_rows, lo_rows, npad = table_layout(lay.n_nodes)
    ec = lay.ec
    e_total = lay.e_total
    n_pad_tot = ec * n_cores - e_total
    nchunk = ec // CHUNK
    nstat = nchunk

    # ---- I/O -------------------------------------------------------------
    eaT = nc.dram_tensor("eaT", [P, ec], bf16, kind="ExternalInput").ap()
    xT = nc.dram_tensor("xT", [P, npad], bf16, kind="ExternalInput").ap()
    srel = nc.dram_tensor("srel", [1, ec], fp16, kind="ExternalInput").ap()
    didx = nc.dram_tensor("didx", [P, ec // 16], i16, kind="ExternalInput").ap()
    iota = nc.dram_tensor("iota", [P, MAXW], f32, kind="ExternalInput").ap()
    wlin = nc.dram_tensor("wlin", [P, P], f32, kind="ExternalInput").ap()
    w1 = nc.dram_tensor("w1", [P, 2 * P], f32, kind="ExternalInput").ap()
    w2 = nc.dram_tensor("w2", [P, P], f32, kind="ExternalInput").ap()
    blin = nc.dram_tensor("blin", [P, 1], f32, kind="ExternalInput").ap()
    g1 = nc.dram_tensor("g1", [P, 1], f32, kind="ExternalInput").ap()
    be1 = nc.dram_tensor("be1", [P, 1], f32, kind="ExternalInput").ap()
    g2 = nc.dram_tensor("g2", [P, 1], f32, kind="ExternalInput").ap()
    be2 = nc.dram_tensor("be2", [P, 1], f32, kind="ExternalInput").ap()
    outT = nc.dram_tensor("outT", [P, ec], bf16, kind="ExternalOutput").ap()

    table = nc.dram_tensor("hw_table", [npad, P], bf16).ap()

    grp_all = [list(range(n_cores))]

    with tile.TileContext(nc) as tc, ExitStack() as es:
        consts = es.enter_context(tc.tile_pool(name="consts", bufs=1))
        gidx = es.enter_context(tc.tile_pool(name="gidx", bufs=4))
        dram = es.enter_context(tc.tile_pool(name="dram", bufs=1, space="DRAM"))
        big = es.enter_context(tc.tile_pool(name="big", bufs=1))
        red = es.enter_context(tc.tile_pool(name="red", bufs=1))

        # ---- warm-up collective (absorbs first-cc setup latency) --------
        wu_in = dram.tile([P, 2], f32, tag="wu_in")
        wu_sb = red.tile([P, 2], f32, tag="wu_sb")
        nc.vector.memset(wu_sb[:], 0.0)
        nc.sync.dma_start(out=wu_in[:], in_=wu_sb[:])
        wu_out = dram.tile([P, 2], f32, tag="wu_out")
        nc.gpsimd.collective_compute(
            "AllReduce", mybir.AluOpType.add, replica_groups=grp_all,
            ins=[wu_in[:].opt()], outs=[wu_out[:].opt()])

        # ---- constants / weight prep ------------------------------------
        ident_f = consts.tile([P, P], f32)
        make_identity(nc, ident_f[:])

        wlin_s = consts.tile([P, P], f32)
        nc.sync.dma_start(out=wlin_s[:], in_=wlin)
        w1_s = consts.tile([P, 2 * P], f32)
        nc.sync.dma_start(out=w1_s[:], in_=w1)
        w2_s = consts.tile([P, P], f32)
        nc.sync.dma_start(out=w2_s[:], in_=w2)
        blin_s = consts.tile([P, 1], f32)
        nc.sync.dma_start(out=blin_s[:], in_=blin)
        g1_s = consts.tile([P, 1], f32)
        nc.sync.dma_start(out=g1_s[:], in_=g1)
        be1_s = consts.tile([P, 1], f32)
        nc.sync.dma_start(out=be1_s[:], in_=be1)
        g2_s = consts.tile([P, 1], f32)
        nc.sync.dma_start(out=g2_s[:], in_=g2)
        be2_s = consts.tile([P, 1], f32)
        nc.sync.dma_start(out=be2_s[:], in_=be2)
        iota_s = consts.tile([P, MAXW], f32)
        nc.sync.dma_start(out=iota_s[:], in_=iota)
        eps_s = consts.tile([P, 1], f32)
        nc.vector.memset(eps_s[:], eps)
        ones16 = consts.tile([1, P], fp16)
        nc.vector.memset(ones16[:], 1.0)

        # prefetch dst idx for the first groups while the table builds
        idx_pre = {}
        for gi, (off, L, _dh) in enumerate(lay.groups[:4]):
            di = gidx.tile([P, GROUP // 16], i16, tag="di")
            nc.sync.dma_start(out=di[:, :L // 16],
                              in_=didx[:, off // 16:(off + L) // 16])
            idx_pre[gi] = di

        w1aT = consts.tile([P, P], f32)
        w1bT = consts.tile([P, P], bf16)
        w2T = consts.tile([P, P], bf16)
        wcT = consts.tile([P, P], bf16)
        bc = consts.tile([P, 1], f32)

        with tc.tile_pool(name="psum0", bufs=1, space="PSUM") as psw, \
             tc.tile_pool(name="psum0b", bufs=3, space="PSUM") as ps0:
            pw = psw.tile([P, P], f32, tag="pw")
            nc.tensor.matmul(pw[:], lhsT=w1_s[:, 0:P], rhs=ident_f[:],
                             start=True, stop=True)
            nc.vector.tensor_copy(w1aT[:], pw[:])
            pw = psw.tile([P, P], f32, tag="pw")
            nc.tensor.matmul(pw[:], lhsT=w1_s[:, P:2 * P], rhs=ident_f[:],
                             start=True, stop=True)
            nc.vector.tensor_copy(w1bT[:], pw[:])
            pw = psw.tile([P, P], f32, tag="pw")
            nc.tensor.matmul(pw[:], lhsT=w2_s[:], rhs=ident_f[:],
                             start=True, stop=True)
            nc.vector.tensor_copy(w2T[:], pw[:])
            # WcT[i, o] = (W1a @ W_lin)[o, i]
            pw = psw.tile([P, P], f32, tag="pw")
            nc.tensor.matmul(pw[:], lhsT=wlin_s[:], rhs=w1aT[:],
                             start=True, stop=True)
            nc.vector.tensor_copy(wcT[:], pw[:])
            pb = psw.tile([P, 1], f32, tag="pb")
            nc.tensor.matmul(pb[:], lhsT=w1aT[:], rhs=blin_s[:],
                             start=True, stop=True)
            nc.vector.tensor_copy(bc[:], pb[:])

            ident_b = consts.tile([P, P], bf16)
            nc.vector.tensor_copy(ident_b[:], ident_f[:])

            # ---- phase 0: build the hW table (hi region first) ----------
            with tc.tile_pool(name="ph0", bufs=4) as ph0:
                zrow = ph0.tile([P, P], bf16, tag="zrow")
                nc.vector.memset(zrow[:], 0.0)

                def build(row0, xcol0, nch):
                    for j in range(nch):
                        xt = ph0.tile([P, 512], bf16, tag="xt")
                        nc.sync.dma_start(
                            out=xt[:],
                            in_=xT[:, xcol0 + j * 512:xcol0 + (j + 1) * 512])
                        hp = ps0.tile([P, 512], f32, tag="hp")
                        nc.tensor.matmul(hp[:], lhsT=wcT[:], rhs=xt[:],
                                         start=True, stop=True)
                        hs = ph0.tile([P, 512], bf16, tag="hs")
                        nc.scalar.activation(hs[:], hp[:], func=FT.Identity,
                                             bias=bc[:], scale=1.0)
                        tp = ps0.tile([P, 512], f32, tag="tp")
                        for s in range(4):
                            nc.tensor.matmul(tp[:, s * P:(s + 1) * P],
                                             lhsT=hs[:, s * P:(s + 1) * P],
                                             rhs=ident_b[:], start=True,
                                             stop=True)
                        ts = ph0.tile([P, 512], bf16, tag="ts")
                        nc.vector.tensor_copy(ts[:], tp[:])
                        r = row0 + j * 512
                        nc.sync.dma_start(
                            out=table[r:r + 512, :].rearrange(
                                "(s p) o -> p s o", p=P),
                            in_=ts[:].rearrange("p (s o) -> p s o", s=4),
                        )

                build(0, 0, hi_rows // 512)
                nc.sync.dma_start(out=table[nhi:nhi + 1, :], in_=zrow[0:1, :])
                build(hi_rows, hi_rows, lo_rows // 512)
                zlo = hi_rows + SPLIT
                nc.sync.dma_start(out=table[zlo:zlo + 1, :], in_=zrow[0:1, :])

        u1 = big.tile([P, ec], bf16)
        stats = consts.tile([P, nstat, 6], f32)

        def bn_coeffs(g_s, be_s, nck, corr=None):
            se = red.tile([P, nstat], f32, tag="se")
            nc.vector.tensor_mul(se[:, :nck], stats[:, :nck, 0],
                                 stats[:, :nck, 1])
            so = red.tile([P, nstat], f32, tag="so")
            nc.vector.tensor_mul(so[:, :nck], stats[:, :nck, 3],
                                 stats[:, :nck, 4])
            qe = red.tile([P, nstat], f32, tag="qe")
            nc.vector.tensor_mul(qe[:, :nck], se[:, :nck], stats[:, :nck, 1])
            nc.vector.tensor_add(qe[:, :nck], qe[:, :nck], stats[:, :nck, 2])
            qo = red.tile([P, nstat], f32, tag="qo")
            nc.vector.tensor_mul(qo[:, :nck], so[:, :nck], stats[:, :nck, 4])
            nc.vector.tensor_add(qo[:, :nck], qo[:, :nck], stats[:, :nck, 5])
            nc.vector.tensor_add(se[:, :nck], se[:, :nck], so[:, :nck])
            nc.vector.tensor_add(qe[:, :nck], qe[:, :nck], qo[:, :nck])
            sq = red.tile([P, 2], f32, tag="sq")
            nc.vector.tensor_reduce(sq[:, 0:1], se[:, :nck],
                                    axis=mybir.AxisListType.X,
                                    op=mybir.AluOpType.add)
            nc.vector.tensor_reduce(sq[:, 1:2], qe[:, :nck],
                                    axis=mybir.AxisListType.X,
                                    op=mybir.AluOpType.add)
            cc_in = dram.tile([P, 2], f32, tag="cc_in")
            nc.sync.dma_start(out=cc_in[:], in_=sq[:])
            cc_out = dram.tile([P, 2], f32, tag="cc_out")
            nc.gpsimd.collective_compute(
                "AllReduce", mybir.AluOpType.add, replica_groups=grp_all,
                ins=[cc_in[:].opt()], outs=[cc_out[:].opt()])
            sqg = red.tile([P, 2], f32, tag="sqg")
            nc.sync.dma_start(out=sqg[:], in_=cc_out[:])
            if corr is not None:
                v, vq = corr
                t = red.tile([P, 2], f32, tag="tcorr")
                nc.vector.tensor_scalar_mul(t[:, 0:1], v[:], float(n_pad_tot))
                nc.vector.tensor_scalar_mul(t[:, 1:2], vq[:], float(n_pad_tot))
                nc.vector.tensor_sub(sqg[:], sqg[:], t[:])
            mu = red.tile([P, 1], f32, tag="mu")
            nc.vector.tensor_scalar_mul(mu[:], sqg[:, 0:1], 1.0 / e_total)
            var = red.tile([P, 1], f32, tag="var")
            nc.vector.tensor_scalar_mul(var[:], sqg[:, 1:2], 1.0 / e_total)
            mu2 = red.tile([P, 1], f32, tag="mu2")
            nc.vector.tensor_mul(mu2[:], mu[:], mu[:])
            nc.vector.tensor_sub(var[:], var[:], mu2[:])
            a = red.tile([P, 1], f32, tag="a")
            nc.scalar.activation(a[:], var[:], func=FT.Sqrt, bias=eps_s[:],
                                 scale=1.0)
            nc.vector.reciprocal(a[:], a[:])
            nc.vector.tensor_mul(a[:], a[:], g_s[:])
            c = red.tile([P, 1], f32, tag="c")
            nc.vector.tensor_mul(c[:], mu[:], a[:])
            nc.vector.tensor_sub(c[:], be_s[:], c[:])
            return a, c

        with (
            tc.tile_pool(name="psA", bufs=4, space="PSUM") as psA,
            tc.tile_pool(name="psB", bufs=2, space="PSUM") as psB,
            tc.tile_pool(name="psS", bufs=1, space="PSUM") as psS,
            tc.tile_pool(name="ea", bufs=2) as eap,
            tc.tile_pool(name="sr", bufs=3) as srp,
            tc.tile_pool(name="gp", bufs=3) as gp,
            tc.tile_pool(name="wr", bufs=3) as wrp,
            tc.tile_pool(name="sp", bufs=3) as Sp,
            tc.tile_pool(name="op", bufs=2) as op,
        ):
            # ---- pass A ------------------------------------------------
            # dst gathers, one per group (queued up front; gp bufs throttle)
            g_tiles = {}
            for gi, (off, L, dst_hi) in enumerate(lay.groups):
                if gi in idx_pre:
                    di = idx_pre[gi]
                else:
                    di = gidx.tile([P, GROUP // 16], i16, tag="di")
                    nc.sync.dma_start(out=di[:, :L // 16],
                                      in_=didx[:, off // 16:(off + L) // 16])
                gdst = gp.tile([P, GROUP], bf16, tag="gdst")
                base = table[0:hi_rows, :] if dst_hi \
                    else table[hi_rows:npad, :]
                nc.gpsimd.dma_gather(
                    out_ap=gdst[:, :L].rearrange("p (a s) -> p a s", a=1),
                    in_ap=base, idxs_ap=di[:, :L // 16],
                    num_idxs=L, num_idxs_reg=L, elem_size=P,
                    transpose=True, single_packet=False)
                g_tiles[gi] = (gdst, off)

            # group-level ea staging
            ea_tiles = {}

            run_tiles = {}
            caps_hi = lay.caps[0]

            for ci, (off, gi, wins) in enumerate(lay.chunks):
                bkt = 0 if off < caps_hi else 1
                goff, gL, _ = lay.groups[gi]
                if gi not in ea_tiles:
                    et = eap.tile([P, GROUP], bf16, tag="ea")
                    nc.sync.dma_start(out=et[:, :gL],
                                      in_=eaT[:, goff:goff + gL])
                    ea_tiles = {gi: et}
                et = ea_tiles[gi]
                rel = off - goff

                up = psA.tile([P, CHUNK], f32, tag="up")
                nc.tensor.matmul(up[:], lhsT=w1bT[:], rhs=et[:, rel:rel + CHUNK],
                                 start=True, stop=(len(wins) == 0),
                                 skip_group_check=bool(wins))

                if wins:
                    st = srp.tile([1, CHUNK], fp16, tag="sr")
                    nc.scalar.dma_start(out=st[:],
                                        in_=srel[:, off:off + CHUNK])
                    bps = psB.tile([P, CHUNK], f32, tag="bps")
                    nc.tensor.matmul(bps[:], lhsT=ones16[:], rhs=st[:],
                                     start=True, stop=True)
                    for wi, (run_i, k, dw, a, b) in enumerate(wins):
                        rkey = (bkt, run_i)
                        if rkey not in run_tiles:
                            wt = wrp.tile([P, RUN], bf16, tag="wt")
                            r0 = lay.runs[run_i]
                            nc.scalar.dma_start(
                                out=wt[:].rearrange("p (k f) -> p k f",
                                                    k=RUN // P),
                                in_=table[r0:r0 + RUN, :].rearrange(
                                    "(k p) f -> p k f", p=P))
                            run_tiles[rkey] = wt
                        wt = run_tiles[rkey]
                        S_w = Sp.tile([P, CHUNK], bf16, tag="S")
                        nc.vector.tensor_scalar(
                            out=S_w[:, a:b], in0=bps[:, a:b],
                            scalar1=iota_s[:, dw:dw + 1], scalar2=None,
                            op0=mybir.AluOpType.is_equal)
                        nc.tensor.matmul(up[:, a:b],
                                         lhsT=wt[:, k * P:(k + 1) * P],
                                         rhs=S_w[:, a:b], start=False,
                                         stop=(wi == len(wins) - 1),
                                         skip_group_check=True)

                gdst, g_off = g_tiles[gi]
                grel = off - g_off
                nc.vector.tensor_add(u1[:, off:off + CHUNK], up[:],
                                     gdst[:, grel:grel + CHUNK])
                nc.vector.bn_stats(stats[:, ci, :], u1[:, off:off + CHUNK])

            a1, c1 = bn_coeffs(g1_s, be1_s, nchunk)

            # pad columns have u1 == 0 -> u2_pad = W2 @ relu(c1), constant
            rc = red.tile([P, 1], f32, tag="rc")
            nc.scalar.activation(rc[:], c1[:], func=FT.Relu)
            rcb = red.tile([P, 1], bf16, tag="rcb")
            nc.vector.tensor_copy(rcb[:], rc[:])
            vp = psS.tile([P, 1], f32, tag="vp")
            nc.tensor.matmul(vp[:], lhsT=w2T[:], rhs=rcb[:],
                             start=True, stop=True)
            v2 = red.tile([P, 1], f32, tag="v2")
            nc.vector.tensor_copy(v2[:], vp[:])
            v2q = red.tile([P, 1], f32, tag="v2q")
            nc.vector.tensor_mul(v2q[:], v2[:], v2[:])

            # ---- pass B: z1 = relu(a1*u1+c1) in place; stats of W2@z1 ---
            for k in range(nchunk):
                off = k * CHUNK
                nc.scalar.activation(u1[:, off:off + CHUNK],
                                     u1[:, off:off + CHUNK],
                                     func=FT.Relu, scale=a1[:], bias=c1[:])
                up = psA.tile([P, CHUNK], f32, tag="up")
                nc.tensor.matmul(up[:], lhsT=w2T[:],
                                 rhs=u1[:, off:off + CHUNK],
                                 start=True, stop=True)
                nc.vector.bn_stats(stats[:, k, :], up[:])

            a2, c2 = bn_coeffs(g2_s, be2_s, nchunk, corr=(v2, v2q))

            # ---- pass C: out = relu(a2*(W2@z1)+c2), staged per group ----
            for base in range(0, ec, GROUP):
                gL = min(GROUP, ec - base)
                ot = op.tile([P, GROUP], bf16, tag="ot")
                for off in range(base, base + gL, CHUNK):
                    up = psA.tile([P, CHUNK], f32, tag="up")
                    nc.tensor.matmul(up[:], lhsT=w2T[:],
                                     rhs=u1[:, off:off + CHUNK],
                                     start=True, stop=True)
                    r = off - base
                    nc.scalar.activation(ot[:, r:r + CHUNK], up[:],
                                         func=FT.Relu, scale=a2[:],
                                         bias=c2[:])
                nc.sync.dma_start(out=outT[:, base:base + gL],
                                  in_=ot[:, :gL])

    nc.compile()
    return nc


def _wrap16(a):
    """linear [L] -> [16, L/16] wrapped, tiled to [128, L/16]."""
    w = np.ascontiguousarray(a.reshape(-1, 16).T)
    return np.tile(w, (8, 1))


def host_prep(x, edge_index, edge_attr, n_cores):
    """Shard, bucket by dst-region, sort by src, build layout + per-core
    arrays."""
    n = x.shape[0]
    e = edge_attr.shape[0]
    ec0 = e // n_cores
    nhi, hi_rows, lo_rows, npad = table_layout(n)

    src_all = edge_index[0].astype(np.int64)
    dst_all = edge_index[1].astype(np.int64)

    per_core = []
    counts = np.zeros((n_cores, 2), np.int64)
    for c in range(n_cores):
        sl = slice(c * ec0, (c + 1) * ec0)
        s, d = src_all[sl], dst_all[sl]
        hi = (d >= SPLIT).astype(np.int64)
        order = np.argsort(hi * (1 << 32) + s, kind="stable")
        counts[c, 1] = int(hi.sum())          # bucket 1 = hi
        counts[c, 0] = ec0 - counts[c, 1]
        per_core.append((s, d, hi, order))

    # bucket order: hi first (table hi region builds first), caps %512
    caps = (int(_r512(counts[:, 1].max())), int(_r512(counts[:, 0].max())))
    ec = caps[0] + caps[1]
    bucket_off = {1: 0, 0: caps[0]}

    zero_hi = nhi                 # local idx of zero row in hi region
    zero_lo = SPLIT               # local idx of zero row in lo region

    # groups (shared across cores: same caps)
    groups = []
    for b, cap in ((1, caps[0]), (0, caps[1])):
        off = bucket_off[b]
        rem = cap
        while rem > 0:
            L = min(GROUP, rem)
            groups.append((off, L, b == 1))
            off += L
            rem -= L

    # per-core padded arrays + union chunk windows
    all_srel = []
    all_didx = []
    all_eacols = []
    all_inv = []
    win_ranges = [dict() for _ in range(ec // CHUNK)]
    w0_arr = np.zeros(ec // CHUNK, np.int64)

    # first pass: compute padded src arrays to derive union windows
    src_p_all = []
    for c in range(n_cores):
        s, d, hi, order = per_core[c]
        src_p = np.full(ec, -1, np.int64)
        dst_p = np.empty(ec, np.int64)
        ea_cols = np.full(ec, -1, np.int64)
        # order sorts by (hi, src): lo bucket first in order, but hi bucket
        # comes first in columns.
        n_lo = int(counts[c, 0])
        idx_lo = order[:n_lo]
        idx_hi = order[n_lo:]
        for b, idx_b in ((1, idx_hi), (0, idx_lo)):
            cnt = len(idx_b)
            off = bucket_off[b]
            pos = off + np.arange(cnt)
            src_p[pos] = s[idx_b]
            dst_p[pos] = d[idx_b] - (SPLIT if b == 1 else 0)
            ea_cols[pos] = idx_b
            padr = np.arange(off + cnt, off + (caps[0] if b == 1 else caps[1]))
            dst_p[padr] = zero_hi if b == 1 else zero_lo
        inv = np.empty(ec0, np.int64)
        inv[idx_hi] = bucket_off[1] + np.arange(len(idx_hi))
        inv[idx_lo] = bucket_off[0] + np.arange(len(idx_lo))
        src_p_all.append(src_p)
        all_didx.append(dst_p)
        all_eacols.append(ea_cols)
        all_inv.append(inv)

        for ci in range(ec // CHUNK):
            seg = src_p[ci * CHUNK:(ci + 1) * CHUNK]
            for w in np.unique(seg[seg >= 0] // P):
                pos = np.nonzero(seg // P == w)[0]
                a, b = int(pos[0]), int(pos[-1]) + 1
                cur = win_ranges[ci].get(int(w))
                if cur is None:
                    win_ranges[ci][int(w)] = [a, b]
                else:
                    cur[0] = min(cur[0], a)
                    cur[1] = max(cur[1], b)

    # per-chunk base window + srel arrays (shared w0 across cores)
    chunks = []
    run_index = {}
    runs = []
    gi_of_off = {off: i for i, (off, L, _) in enumerate(groups)
                 for off in range(off, off + L, CHUNK)}
    for ci in range(ec // CHUNK):
        wins = sorted(win_ranges[ci])
        off = ci * CHUNK
        wl = []
        if wins:
            w0 = wins[0]
            w0_arr[ci] = w0
            assert wins[-1] - w0 < MAXW, f"chunk {ci} spans {wins}"
            for w in wins:
                node0 = w * P
                if node0 >= SPLIT:
                    row0 = node0 - SPLIT          # hi region
                else:
                    row0 = hi_rows + node0        # lo region
                run0 = (row0 // RUN) * RUN
                if run0 not in run_index:
                    run_index[run0] = len(runs)
                    runs.append(run0)
                a, b = win_ranges[ci][w]
                wl.append((run_index[run0], (row0 - run0) // P, w - w0,
                           a, b))
        chunks.append((off, gi_of_off[off], wl))

    for c in range(n_cores):
        src_p = src_p_all[c]
        srel = np.full(ec, -1.0, np.float32)
        for ci in range(ec // CHUNK):
            seg = src_p[ci * CHUNK:(ci + 1) * CHUNK]
            m = seg >= 0
            srel[ci * CHUNK:(ci + 1) * CHUNK][m] = seg[m] - w0_arr[ci] * P
        assert srel.max() < 2048
        all_srel.append(srel.astype(FP16))

    lay = Layout(caps, chunks, groups, runs, n, e)
    return lay, all_srel, all_didx, all_eacols, all_inv


def make_in_maps(x, edge_index, edge_attr, W_lin, b_lin, W1, g1, be1, W2,
                 g2, be2, n_cores):
    n = x.shape[0]
    nhi, hi_rows, lo_rows, npad = table_layout(n)
    lay, all_srel, all_didx, all_eacols, all_inv = host_prep(
        x, edge_index, edge_attr, n_cores)
    ec = lay.ec
    ec0 = edge_attr.shape[0] // n_cores

    xbf = x.astype(BF16)
    xT = np.zeros((P, npad), dtype=BF16)
    xT[:, 0:nhi] = xbf[SPLIT:n].T
    xT[:, hi_rows:hi_rows + SPLIT] = xbf[0:SPLIT].T

    iota = (np.arange(P)[:, None]
            + P * np.arange(MAXW)[None, :]).astype(np.float32)

    f32c = np.ascontiguousarray
    wlin_h = f32c(W_lin.astype(np.float32))
    w1_h = f32c(W1.astype(np.float32))
    w2_h = f32c(W2.astype(np.float32))
    blin_h = f32c(b_lin.astype(np.float32).reshape(P, 1))
    g1_h = f32c(g1.astype(np.float32).reshape(P, 1))
    be1_h = f32c(be1.astype(np.float32).reshape(P, 1))
    g2_h = f32c(g2.astype(np.float32).reshape(P, 1))
    be2_h = f32c(be2.astype(np.float32).reshape(P, 1))

    eabf = edge_attr.astype(BF16)

    in_maps = []
    for c in range(n_cores):
        ea_cols = all_eacols[c]
        eaT = np.zeros((P, ec), dtype=BF16)
        real = ea_cols >= 0
        eaT[:, real] = eabf[c * ec0 + ea_cols[real]].T
        dw = np.zeros((P, ec // 16), np.int16)
        for off, L, _ in lay.groups:
            dw[:, off // 16:(off + L) // 16] = _wrap16(
                all_didx[c][off:off + L].astype(np.int16))
        in_maps.append({
            "eaT": eaT, "xT": xT, "srel": all_srel[c].reshape(1, ec),
            "didx": dw, "iota": iota,
            "wlin": wlin_h, "w1": w1_h, "w2": w2_h, "blin": blin_h,
            "g1": g1_h, "be1": be1_h, "g2": g2_h, "be2": be2_h,
        })
    return lay, in_maps, all_inv


_GRAPH_CACHE = {}


def get_graph(lay: Layout):
    if lay.key not in _GRAPH_CACHE:
        _GRAPH_CACHE[lay.key] = build_graph(lay)
    return _GRAPH_CACHE[lay.key]


def kernel(x, edge_index, edge_attr, W_lin, b_lin, W1, b1, g1, be1, W2, b2,
           g2, be2):
    """Full-input entry point: shard, run on 8 NeuronCores, gather."""
    x = np.asarray(x)
    edge_index = np.asarray(edge_index)
    edge_attr = np.asarray(edge_attr)
    e = edge_attr.shape[0]
    ec0 = e // N_CORES

    lay, in_maps, invs = make_in_maps(
        x, edge_index, edge_attr, np.asarray(W_lin), np.asarray(b_lin),
        np.asarray(W1), np.asarray(g1), np.asarray(be1), np.asarray(W2),
        np.asarray(g2), np.asarray(be2), N_CORES)
    nc = get_graph(lay)
    res = run_bass_kernel_spmd(nc, in_maps, core_ids=list(range(N_CORES)))
    out = np.empty((e, NIN), dtype=np.float32)
    for c in range(N_CORES):
        oT = np.asarray(res.results[c]["outT"], dtype=np.float32)
        out[c * ec0:(c + 1) * ec0] = oT.T[invs[c]]
    return out


# revision 18
# speedup vs baseline: 1.0062x; 1.0016x over previous
"""Trainium2 Bass kernel for the GNN edge-update MLP (8 NeuronCores).

Reference semantics:
    h   = x @ W_lin.T + b_lin                       # [N, nin]
    agg = h[src] + h[dst]                           # [E, nin]
    z   = concat([agg, edge_attr], -1)              # [E, 2*nin]
    z   = relu(BN(z @ W1.T + b1; g1, be1))          # [E, nout]  (BN over edges)
    z   = relu(BN(z @ W2.T + b2; g2, be2))          # [E, nout]

Restructuring (v2 — one-hot src path):
  * b1/b2 cancel inside training-mode BN -> dropped.
  * z @ W1.T = hW[src] + hW[dst] + ea @ W1b.T, with W1 = [W1a | W1b] and
    hW = x @ (W1a W_lin).T + W1a b_lin  (a [N, nout] row-major DRAM table).
  * Edges are sharded over 8 cores; per core they are bucketed by
    (dst >= SPLIT) [int16 gather trick] and SORTED BY SRC inside each bucket.
  * dst contribution: GPSIMD dma_gather from the table (per-edge descriptors,
    ~8ns each — this is the only per-edge SWDGE cost left).
  * src contribution: because edges are src-sorted, each 512-column chunk's
    srcs span only ~2-4 aligned 128-node windows.  For each window the PE
    multiplies the table slice (lhsT [128 nodes, 128 feat]) by a one-hot
    selection matrix S (built on DVE by comparing a broadcast src-value row
    against an iota column) and accumulates into the chunk's PSUM bank.
    No per-edge descriptors at all.
  * The per-chunk window structure (and bucket caps) are data-dependent and
    baked into the compiled graph (compile happens per input set; cached).
  * BN statistics: per-chunk vector bn_stats, merged, AllReduce'd ([128,2]).
    A dummy AllReduce at t=0 absorbs the first-collective setup cost.
  * Padded columns are exactly zero through u1 (src matches no window row,
    dst gathers a zero row, ea is zero); their constant effect on the second
    BN is subtracted analytically (v2 correction), as in v1.
"""

import sys
from contextlib import ExitStack

import numpy as np

try:
    import concourse  # noqa: F401
except ImportError:  # pragma: no cover
    sys.path.insert(0, "/opt/trn_rl_repo")

import ml_dtypes
from concourse import bass, bacc, mybir
from concourse import tile
from concourse.bass_utils import run_bass_kernel_spmd
from concourse.masks import make_identity

BF16 = ml_dtypes.bfloat16
FP16 = np.float16

N_CORES = 8
NIN = 128
EPS = 1e-5
P = 128

SPLIT = 32640            # nodes < SPLIT are "lo", >= SPLIT are "hi" (128-mult)
GROUP = 2048             # edges per dma_gather instruction
CHUNK = 512
RUN = 512                # table rows per window-run load (4 windows)
MAXW = 16                # max windows (of 128 rows) spanned by one chunk


def _r512(v):
    return ((v + 511) // 512) * 512


def table_layout(n_nodes):
    """One DRAM table, hi region first (rows [0, hi_rows)), then lo."""
    nhi = n_nodes - SPLIT
    hi_rows = _r512(nhi + 1)
    lo_rows = _r512(SPLIT + 1)
    return nhi, hi_rows, lo_rows, hi_rows + lo_rows


class Layout:
    """Per-compile structural data (hashable via .key)."""

    def __init__(self, caps, chunks, groups, runs, n_nodes, e_total):
        self.caps = caps          # (cap_hi, cap_lo) in BUCKET order (hi, lo)
        self.chunks = chunks      # [(off, gi, [(run_i, k, dw), ...])]
        self.groups = groups      # [(off, L, dst_hi)]
        self.runs = runs          # [table_row0] per run (RUN rows each)
        self.n_nodes = n_nodes
        self.e_total = e_total
        self.ec = sum(caps)
        self.key = (caps, tuple((o, g, tuple(w)) for o, g, w in chunks),
                    tuple(groups), tuple(runs), n_nodes, e_total)


def build_graph(lay: Layout, n_cores=N_CORES, eps=EPS):
    f32 = mybir.dt.float32
    bf16 = mybir.dt.bfloat16
    fp16 = mybir.dt.float16
    f8 = mybir.dt.float8e4
    i16 = mybir.dt.int16
    FT = mybir.ActivationFunctionType

    nc = bacc.Bacc(
        "TRN2", target_bir_lowering=False, debug=False, num_devices=n_cores
    )

    nhi, hi_rows, lo_rows, npad = table_layout(lay.n_nodes)
    ec = lay.ec
    e_total = lay.e_total
    n_pad_tot = ec * n_cores - e_total
    nchunk = ec // CHUNK
    nstat = nchunk

    # ---- I/O -------------------------------------------------------------
    eaT = nc.dram_tensor("eaT", [P, ec], bf16, kind="ExternalInput").ap()
    xT = nc.dram_tensor("xT", [P, npad], bf16, kind="ExternalInput").ap()
    srel = nc.dram_tensor("srel", [1, ec], fp16, kind="ExternalInput").ap()
    didx = nc.dram_tensor("didx", [P, ec // 16], i16, kind="ExternalInput").ap()
    iota = nc.dram_tensor("iota", [P, MAXW], f32, kind="ExternalInput").ap()
    wlin = nc.dram_tensor("wlin", [P, P], f32, kind="ExternalInput").ap()
    w1 = nc.dram_tensor("w1", [P, 2 * P], f32, kind="ExternalInput").ap()
    w2 = nc.dram_tensor("w2", [P, P], f32, kind="ExternalInput").ap()
    blin = nc.dram_tensor("blin", [P, 1], f32, kind="ExternalInput").ap()
    g1 = nc.dram_tensor("g1", [P, 1], f32, kind="ExternalInput").ap()
    be1 = nc.dram_tensor("be1", [P, 1], f32, kind="ExternalInput").ap()
    g2 = nc.dram_tensor("g2", [P, 1], f32, kind="ExternalInput").ap()
    be2 = nc.dram_tensor("be2", [P, 1], f32, kind="ExternalInput").ap()
    outT = nc.dram_tensor("outT", [P, ec], bf16, kind="ExternalOutput").ap()

    table = nc.dram_tensor("hw_table", [npad, P], bf16).ap()

    grp_all = [list(range(n_cores))]

    with tile.TileContext(nc) as tc, ExitStack() as es:
        consts = es.enter_context(tc.tile_pool(name="consts", bufs=1))
        gidx = es.enter_context(tc.tile_pool(name="gidx", bufs=4))
        dram = es.enter_context(tc.tile_pool(name="dram", bufs=1, space="DRAM"))
        big = es.enter_context(tc.tile_pool(name="big", bufs=1))
        red = es.enter_context(tc.tile_pool(name="red", bufs=1))

        # ---- warm-up collective (absorbs first-cc setup latency) --------
        wu_in = dram.tile([P, 2], f32, tag="wu_in")
        wu_sb = red.tile([P, 2], f32, tag="wu_sb")
        nc.vector.memset(wu_sb[:], 0.0)
        nc.sync.dma_start(out=wu_in[:], in_=wu_sb[:])
        wu_out = dram.tile([P, 2], f32, tag="wu_out")
        nc.gpsimd.collective_compute(
            "AllReduce", mybir.AluOpType.add, replica_groups=grp_all,
            ins=[wu_in[:].opt()], outs=[wu_out[:].opt()])

        # ---- constants / weight prep ------------------------------------
        ident_f = consts.tile([P, P], f32)
        make_identity(nc, ident_f[:])

        wlin_s = consts.tile([P, P], f32)
        nc.sync.dma_start(out=wlin_s[:], in_=wlin)
        w1_s = consts.tile([P, 2 * P], f32)
        nc.sync.dma_start(out=w1_s[:], in_=w1)
        w2_s = consts.tile([P, P], f32)
        nc.sync.dma_start(out=w2_s[:], in_=w2)
        blin_s = consts.tile([P, 1], f32)
        nc.sync.dma_start(out=blin_s[:], in_=blin)
        g1_s = consts.tile([P, 1], f32)
        nc.sync.dma_start(out=g1_s[:], in_=g1)
        be1_s = consts.tile([P, 1], f32)
        nc.sync.dma_start(out=be1_s[:], in_=be1)
        g2_s = consts.tile([P, 1], f32)
        nc.sync.dma_start(out=g2_s[:], in_=g2)
        be2_s = consts.tile([P, 1], f32)
        nc.sync.dma_start(out=be2_s[:], in_=be2)
        iota_s = consts.tile([P, MAXW], f32)
        nc.sync.dma_start(out=iota_s[:], in_=iota)
        eps_s = consts.tile([P, 1], f32)
        nc.vector.memset(eps_s[:], eps)
        ones16 = consts.tile([1, P], fp16)
        nc.vector.memset(ones16[:], 1.0)

        # prefetch dst idx for the first groups while the table builds
        idx_pre = {}
        for gi, (off, L, _dh) in enumerate(lay.groups[:4]):
            di = gidx.tile([P, GROUP // 16], i16, tag="di")
            nc.sync.dma_start(out=di[:, :L // 16],
                              in_=didx[:, off // 16:(off + L) // 16])
            idx_pre[gi] = di

        w1aT = consts.tile([P, P], f32)
        w1bT = consts.tile([P, P], bf16)
        w2T = consts.tile([P, P], bf16)
        wcT = consts.tile([P, P], bf16)
        bc = consts.tile([P, 1], f32)

        with tc.tile_pool(name="psum0", bufs=1, space="PSUM") as psw, \
             tc.tile_pool(name="psum0b", bufs=3, space="PSUM") as ps0:
            pw = psw.tile([P, P], f32, tag="pw")
            nc.tensor.matmul(pw[:], lhsT=w1_s[:, 0:P], rhs=ident_f[:],
                             start=True, stop=True)
            nc.vector.tensor_copy(w1aT[:], pw[:])
            pw = psw.tile([P, P], f32, tag="pw")
            nc.tensor.matmul(pw[:], lhsT=w1_s[:, P:2 * P], rhs=ident_f[:],
                             start=True, stop=True)
            nc.vector.tensor_copy(w1bT[:], pw[:])
            pw = psw.tile([P, P], f32, tag="pw")
            nc.tensor.matmul(pw[:], lhsT=w2_s[:], rhs=ident_f[:],
                             start=True, stop=True)
            nc.vector.tensor_copy(w2T[:], pw[:])
            # WcT[i, o] = (W1a @ W_lin)[o, i]
            pw = psw.tile([P, P], f32, tag="pw")
            nc.tensor.matmul(pw[:], lhsT=wlin_s[:], rhs=w1aT[:],
                             start=True, stop=True)
            nc.vector.tensor_copy(wcT[:], pw[:])
            pb = psw.tile([P, 1], f32, tag="pb")
            nc.tensor.matmul(pb[:], lhsT=w1aT[:], rhs=blin_s[:],
                             start=True, stop=True)
            nc.vector.tensor_copy(bc[:], pb[:])

            ident_b = consts.tile([P, P], bf16)
            nc.vector.tensor_copy(ident_b[:], ident_f[:])

            # ---- phase 0: build the hW table (hi region first) ----------
            with tc.tile_pool(name="ph0", bufs=4) as ph0:
                zrow = ph0.tile([P, P], bf16, tag="zrow")
                nc.vector.memset(zrow[:], 0.0)

                def build(row0, xcol0, nch):
                    for j in range(nch):
                        xt = ph0.tile([P, 512], bf16, tag="xt")
                        nc.sync.dma_start(
                            out=xt[:],
                            in_=xT[:, xcol0 + j * 512:xcol0 + (j + 1) * 512])
                        hp = ps0.tile([P, 512], f32, tag="hp")
                        nc.tensor.matmul(hp[:], lhsT=wcT[:], rhs=xt[:],
                                         start=True, stop=True)
                        hs = ph0.tile([P, 512], bf16, tag="hs")
                        nc.scalar.activation(hs[:], hp[:], func=FT.Identity,
                                             bias=bc[:], scale=1.0)
                        tp = ps0.tile([P, 512], f32, tag="tp")
                        for s in range(4):
                            nc.tensor.matmul(tp[:, s * P:(s + 1) * P],
                                             lhsT=hs[:, s * P:(s + 1) * P],
                                             rhs=ident_b[:], start=True,
                                             stop=True)
                        ts = ph0.tile([P, 512], bf16, tag="ts")
                        nc.vector.tensor_copy(ts[:], tp[:])
                        r = row0 + j * 512
                        nc.sync.dma_start(
                            out=table[r:r + 512, :].rearrange(
                                "(s p) o -> p s o", p=P),
                            in_=ts[:].rearrange("p (s o) -> p s o", s=4),
                        )

                build(0, 0, hi_rows // 512)
                nc.sync.dma_start(out=table[nhi:nhi + 1, :], in_=zrow[0:1, :])
                build(hi_rows, hi_rows, lo_rows // 512)
                zlo = hi_rows + SPLIT
                nc.sync.dma_start(out=table[zlo:zlo + 1, :], in_=zrow[0:1, :])

        u1 = big.tile([P, ec], bf16)
        stats = consts.tile([P, nstat, 6], f32)

        def bn_coeffs(g_s, be_s, nck, corr=None):
            se = red.tile([P, nstat], f32, tag="se")
            nc.vector.tensor_mul(se[:, :nck], stats[:, :nck, 0],
                                 stats[:, :nck, 1])
            so = red.tile([P, nstat], f32, tag="so")
            nc.vector.tensor_mul(so[:, :nck], stats[:, :nck, 3],
                                 stats[:, :nck, 4])
            qe = red.tile([P, nstat], f32, tag="qe")
            nc.vector.tensor_mul(qe[:, :nck], se[:, :nck], stats[:, :nck, 1])
            nc.vector.tensor_add(qe[:, :nck], qe[:, :nck], stats[:, :nck, 2])
            qo = red.tile([P, nstat], f32, tag="qo")
            nc.vector.tensor_mul(qo[:, :nck], so[:, :nck], stats[:, :nck, 4])
            nc.vector.tensor_add(qo[:, :nck], qo[:, :nck], stats[:, :nck, 5])
            nc.vector.tensor_add(se[:, :nck], se[:, :nck], so[:, :nck])
            nc.vector.tensor_add(qe[:, :nck], qe[:, :nck], qo[:, :nck])
            sq = red.tile([P, 2], f32, tag="sq")
            nc.vector.tensor_reduce(sq[:, 0:1], se[:, :nck],
                                    axis=mybir.AxisListType.X,
                                    op=mybir.AluOpType.add)
            nc.vector.tensor_reduce(sq[:, 1:2], qe[:, :nck],
                                    axis=mybir.AxisListType.X,
                                    op=mybir.AluOpType.add)
            cc_in = dram.tile([P, 2], f32, tag="cc_in")
            nc.sync.dma_start(out=cc_in[:], in_=sq[:])
            cc_out = dram.tile([P, 2], f32, tag="cc_out")
            nc.gpsimd.collective_compute(
                "AllReduce", mybir.AluOpType.add, replica_groups=grp_all,
                ins=[cc_in[:].opt()], outs=[cc_out[:].opt()])
            sqg = red.tile([P, 2], f32, tag="sqg")
            nc.sync.dma_start(out=sqg[:], in_=cc_out[:])
            if corr is not None:
                v, vq = corr
                t = red.tile([P, 2], f32, tag="tcorr")
                nc.vector.tensor_scalar_mul(t[:, 0:1], v[:], float(n_pad_tot))
                nc.vector.tensor_scalar_mul(t[:, 1:2], vq[:], float(n_pad_tot))
                nc.vector.tensor_sub(sqg[:], sqg[:], t[:])
            mu = red.tile([P, 1], f32, tag="mu")
            nc.vector.tensor_scalar_mul(mu[:], sqg[:, 0:1], 1.0 / e_total)
            var = red.tile([P, 1], f32, tag="var")
            nc.vector.tensor_scalar_mul(var[:], sqg[:, 1:2], 1.0 / e_total)
            mu2 = red.tile([P, 1], f32, tag="mu2")
            nc.vector.tensor_mul(mu2[:], mu[:], mu[:])
            nc.vector.tensor_sub(var[:], var[:], mu2[:])
            a = red.tile([P, 1], f32, tag="a")
            nc.scalar.activation(a[:], var[:], func=FT.Sqrt, bias=eps_s[:],
                                 scale=1.0)
            nc.vector.reciprocal(a[:], a[:])
            nc.vector.tensor_mul(a[:], a[:], g_s[:])
            c = red.tile([P, 1], f32, tag="c")
            nc.vector.tensor_mul(c[:], mu[:], a[:])
            nc.vector.tensor_sub(c[:], be_s[:], c[:])
            return a, c

        with (
            tc.tile_pool(name="psA", bufs=4, space="PSUM") as psA,
            tc.tile_pool(name="psB", bufs=2, space="PSUM") as psB,
            tc.tile_pool(name="psS", bufs=1, space="PSUM") as psS,
            tc.tile_pool(name="ea", bufs=2) as eap,
            tc.tile_pool(name="sr", bufs=3) as srp,
            tc.tile_pool(name="gp", bufs=3) as gp,
            tc.tile_pool(name="wr", bufs=3) as wrp,
            tc.tile_pool(name="sp", bufs=3) as Sp,
            tc.tile_pool(name="op", bufs=2) as op,
        ):
            # ---- pass A ------------------------------------------------
            # dst gathers, one per group (queued up front; gp bufs throttle)
            g_tiles = {}
            for gi, (off, L, dst_hi) in enumerate(lay.groups):
                if gi in idx_pre:
                    di = idx_pre[gi]
                else:
                    di = gidx.tile([P, GROUP // 16], i16, tag="di")
                    nc.sync.dma_start(out=di[:, :L // 16],
                                      in_=didx[:, off // 16:(off + L) // 16])
                gdst = gp.tile([P, GROUP], bf16, tag="gdst")
                base = table[0:hi_rows, :] if dst_hi \
                    else table[hi_rows:npad, :]
                nc.gpsimd.dma_gather(
                    out_ap=gdst[:, :L].rearrange("p (a s) -> p a s", a=1),
                    in_ap=base, idxs_ap=di[:, :L // 16],
                    num_idxs=L, num_idxs_reg=L, elem_size=P,
                    transpose=True, single_packet=False)
                g_tiles[gi] = (gdst, off)

            # group-level ea staging
            ea_tiles = {}

            run_tiles = {}
            caps_hi = lay.caps[0]

            for ci, (off, gi, wins) in enumerate(lay.chunks):
                bkt = 0 if off < caps_hi else 1
                goff, gL, _ = lay.groups[gi]
                if gi not in ea_tiles:
                    et = eap.tile([P, GROUP], bf16, tag="ea")
                    nc.sync.dma_start(out=et[:, :gL],
                                      in_=eaT[:, goff:goff + gL])
                    ea_tiles = {gi: et}
                et = ea_tiles[gi]
                rel = off - goff

                up = psA.tile([P, CHUNK], f32, tag="up")
                nc.tensor.matmul(up[:], lhsT=w1bT[:], rhs=et[:, rel:rel + CHUNK],
                                 start=True, stop=(len(wins) == 0),
                                 skip_group_check=bool(wins))

                if wins:
                    clo = min(w[3] for w in wins)
                    chi = max(w[4] for w in wins)
                    st = srp.tile([1, CHUNK], fp16, tag="sr")
                    nc.scalar.dma_start(out=st[:, clo:chi],
                                        in_=srel[:, off + clo:off + chi])
                    bps = psB.tile([P, CHUNK], f32, tag="bps")
                    nc.tensor.matmul(bps[:, clo:chi], lhsT=ones16[:],
                                     rhs=st[:, clo:chi],
                                     start=True, stop=True)
                    for wi, (run_i, k, dw, a, b) in enumerate(wins):
                        rkey = (bkt, run_i)
                        if rkey not in run_tiles:
                            wt = wrp.tile([P, RUN], bf16, tag="wt")
                            r0 = lay.runs[run_i]
                            nc.scalar.dma_start(
                                out=wt[:].rearrange("p (k f) -> p k f",
                                                    k=RUN // P),
                                in_=table[r0:r0 + RUN, :].rearrange(
                                    "(k p) f -> p k f", p=P))
                            run_tiles[rkey] = wt
                        wt = run_tiles[rkey]
                        S_w = Sp.tile([P, CHUNK], bf16, tag="S")
                        nc.vector.tensor_scalar(
                            out=S_w[:, a:b], in0=bps[:, a:b],
                            scalar1=iota_s[:, dw:dw + 1], scalar2=None,
                            op0=mybir.AluOpType.is_equal)
                        nc.tensor.matmul(up[:, a:b],
                                         lhsT=wt[:, k * P:(k + 1) * P],
                                         rhs=S_w[:, a:b], start=False,
                                         stop=(wi == len(wins) - 1),
                                         skip_group_check=True)

                gdst, g_off = g_tiles[gi]
                grel = off - g_off
                nc.vector.tensor_add(u1[:, off:off + CHUNK], up[:],
                                     gdst[:, grel:grel + CHUNK])
                nc.vector.bn_stats(stats[:, ci, :], u1[:, off:off + CHUNK])

            a1, c1 = bn_coeffs(g1_s, be1_s, nchunk)

            # pad columns have u1 == 0 -> u2_pad = W2 @ relu(c1), constant
            rc = red.tile([P, 1], f32, tag="rc")
            nc.scalar.activation(rc[:], c1[:], func=FT.Relu)
            rcb = red.tile([P, 1], bf16, tag="rcb")
            nc.vector.tensor_copy(rcb[:], rc[:])
            vp = psS.tile([P, 1], f32, tag="vp")
            nc.tensor.matmul(vp[:], lhsT=w2T[:], rhs=rcb[:],
                             start=True, stop=True)
            v2 = red.tile([P, 1], f32, tag="v2")
            nc.vector.tensor_copy(v2[:], vp[:])
            v2q = red.tile([P, 1], f32, tag="v2q")
            nc.vector.tensor_mul(v2q[:], v2[:], v2[:])

            # ---- pass B: z1 = relu(a1*u1+c1) in place; stats of W2@z1 ---
            for k in range(nchunk):
                off = k * CHUNK
                nc.scalar.activation(u1[:, off:off + CHUNK],
                                     u1[:, off:off + CHUNK],
                                     func=FT.Relu, scale=a1[:], bias=c1[:])
                up = psA.tile([P, CHUNK], f32, tag="up")
                nc.tensor.matmul(up[:], lhsT=w2T[:],
                                 rhs=u1[:, off:off + CHUNK],
                                 start=True, stop=True)
                nc.vector.bn_stats(stats[:, k, :], up[:])

            a2, c2 = bn_coeffs(g2_s, be2_s, nchunk, corr=(v2, v2q))

            # ---- pass C: out = relu(a2*(W2@z1)+c2), staged per group ----
            for base in range(0, ec, GROUP):
                gL = min(GROUP, ec - base)
                ot = op.tile([P, GROUP], bf16, tag="ot")
                for off in range(base, base + gL, CHUNK):
                    up = psA.tile([P, CHUNK], f32, tag="up")
                    nc.tensor.matmul(up[:], lhsT=w2T[:],
                                     rhs=u1[:, off:off + CHUNK],
                                     start=True, stop=True)
                    r = off - base
                    nc.scalar.activation(ot[:, r:r + CHUNK], up[:],
                                         func=FT.Relu, scale=a2[:],
                                         bias=c2[:])
                nc.sync.dma_start(out=outT[:, base:base + gL],
                                  in_=ot[:, :gL])

    nc.compile()
    return nc


def _wrap16(a):
    """linear [L] -> [16, L/16] wrapped, tiled to [128, L/16]."""
    w = np.ascontiguousarray(a.reshape(-1, 16).T)
    return np.tile(w, (8, 1))


def host_prep(x, edge_index, edge_attr, n_cores):
    """Shard, bucket by dst-region, sort by src, build layout + per-core
    arrays."""
    n = x.shape[0]
    e = edge_attr.shape[0]
    ec0 = e // n_cores
    nhi, hi_rows, lo_rows, npad = table_layout(n)

    src_all = edge_index[0].astype(np.int64)
    dst_all = edge_index[1].astype(np.int64)

    per_core = []
    counts = np.zeros((n_cores, 2), np.int64)
    for c in range(n_cores):
        sl = slice(c * ec0, (c + 1) * ec0)
        s, d = src_all[sl], dst_all[sl]
        hi = (d >= SPLIT).astype(np.int64)
        order = np.argsort(hi * (1 << 32) + s, kind="stable")
        counts[c, 1] = int(hi.sum())          # bucket 1 = hi
        counts[c, 0] = ec0 - counts[c, 1]
        per_core.append((s, d, hi, order))

    # bucket order: hi first (table hi region builds first), caps %512
    caps = (int(_r512(counts[:, 1].max())), int(_r512(counts[:, 0].max())))
    ec = caps[0] + caps[1]
    bucket_off = {1: 0, 0: caps[0]}

    zero_hi = nhi                 # local idx of zero row in hi region
    zero_lo = SPLIT               # local idx of zero row in lo region

    # groups (shared across cores: same caps)
    groups = []
    for b, cap in ((1, caps[0]), (0, caps[1])):
        off = bucket_off[b]
        rem = cap
        while rem > 0:
            L = min(GROUP, rem)
            groups.append((off, L, b == 1))
            off += L
            rem -= L

    # per-core padded arrays + union chunk windows
    all_srel = []
    all_didx = []
    all_eacols = []
    all_inv = []
    win_ranges = [dict() for _ in range(ec // CHUNK)]
    w0_arr = np.zeros(ec // CHUNK, np.int64)

    # first pass: compute padded src arrays to derive union windows
    src_p_all = []
    for c in range(n_cores):
        s, d, hi, order = per_core[c]
        src_p = np.full(ec, -1, np.int64)
        dst_p = np.empty(ec, np.int64)
        ea_cols = np.full(ec, -1, np.int64)
        # order sorts by (hi, src): lo bucket first in order, but hi bucket
        # comes first in columns.
        n_lo = int(counts[c, 0])
        idx_lo = order[:n_lo]
        idx_hi = order[n_lo:]
        for b, idx_b in ((1, idx_hi), (0, idx_lo)):
            cnt = len(idx_b)
            off = bucket_off[b]
            pos = off + np.arange(cnt)
            src_p[pos] = s[idx_b]
            dst_p[pos] = d[idx_b] - (SPLIT if b == 1 else 0)
            ea_cols[pos] = idx_b
            padr = np.arange(off + cnt, off + (caps[0] if b == 1 else caps[1]))
            dst_p[padr] = zero_hi if b == 1 else zero_lo
        inv = np.empty(ec0, np.int64)
        inv[idx_hi] = bucket_off[1] + np.arange(len(idx_hi))
        inv[idx_lo] = bucket_off[0] + np.arange(len(idx_lo))
        src_p_all.append(src_p)
        all_didx.append(dst_p)
        all_eacols.append(ea_cols)
        all_inv.append(inv)

        for ci in range(ec // CHUNK):
            seg = src_p[ci * CHUNK:(ci + 1) * CHUNK]
            for w in np.unique(seg[seg >= 0] // P):
                pos = np.nonzero(seg // P == w)[0]
                a, b = int(pos[0]), int(pos[-1]) + 1
                cur = win_ranges[ci].get(int(w))
                if cur is None:
                    win_ranges[ci][int(w)] = [a, b]
                else:
                    cur[0] = min(cur[0], a)
                    cur[1] = max(cur[1], b)

    # per-chunk base window + srel arrays (shared w0 across cores)
    chunks = []
    run_index = {}
    runs = []
    gi_of_off = {off: i for i, (off, L, _) in enumerate(groups)
                 for off in range(off, off + L, CHUNK)}
    for ci in range(ec // CHUNK):
        wins = sorted(win_ranges[ci])
        off = ci * CHUNK
        wl = []
        if wins:
            w0 = wins[0]
            w0_arr[ci] = w0
            assert wins[-1] - w0 < MAXW, f"chunk {ci} spans {wins}"
            for w in wins:
                node0 = w * P
                if node0 >= SPLIT:
                    row0 = node0 - SPLIT          # hi region
                else:
                    row0 = hi_rows + node0        # lo region
                run0 = (row0 // RUN) * RUN
                if run0 not in run_index:
                    run_index[run0] = len(runs)
                    runs.append(run0)
                a, b = win_ranges[ci][w]
                wl.append((run_index[run0], (row0 - run0) // P, w - w0,
                           a, b))
        chunks.append((off, gi_of_off[off], wl))

    for c in range(n_cores):
        src_p = src_p_all[c]
        srel = np.full(ec, -1.0, np.float32)
        for ci in range(ec // CHUNK):
            seg = src_p[ci * CHUNK:(ci + 1) * CHUNK]
            m = seg >= 0
            srel[ci * CHUNK:(ci + 1) * CHUNK][m] = seg[m] - w0_arr[ci] * P
        assert srel.max() < 2048
        all_srel.append(srel.astype(FP16))

    lay = Layout(caps, chunks, groups, runs, n, e)
    return lay, all_srel, all_didx, all_eacols, all_inv


def make_in_maps(x, edge_index, edge_attr, W_lin, b_lin, W1, g1, be1, W2,
                 g2, be2, n_cores):
    n = x.shape[0]
    nhi, hi_rows, lo_rows, npad = table_layout(n)
    lay, all_srel, all_didx, all_eacols, all_inv = host_prep(
        x, edge_index, edge_attr, n_cores)
    ec = lay.ec
    ec0 = edge_attr.shape[0] // n_cores

    xbf = x.astype(BF16)
    xT = np.zeros((P, npad), dtype=BF16)
    xT[:, 0:nhi] = xbf[SPLIT:n].T
    xT[:, hi_rows:hi_rows + SPLIT] = xbf[0:SPLIT].T

    iota = (np.arange(P)[:, None]
            + P * np.arange(MAXW)[None, :]).astype(np.float32)

    f32c = np.ascontiguousarray
    wlin_h = f32c(W_lin.astype(np.float32))
    w1_h = f32c(W1.astype(np.float32))
    w2_h = f32c(W2.astype(np.float32))
    blin_h = f32c(b_lin.astype(np.float32).reshape(P, 1))
    g1_h = f32c(g1.astype(np.float32).reshape(P, 1))
    be1_h = f32c(be1.astype(np.float32).reshape(P, 1))
    g2_h = f32c(g2.astype(np.float32).reshape(P, 1))
    be2_h = f32c(be2.astype(np.float32).reshape(P, 1))

    eabf = edge_attr.astype(BF16)

    in_maps = []
    for c in range(n_cores):
        ea_cols = all_eacols[c]
        eaT = np.zeros((P, ec), dtype=BF16)
        real = ea_cols >= 0
        eaT[:, real] = eabf[c * ec0 + ea_cols[real]].T
        dw = np.zeros((P, ec // 16), np.int16)
        for off, L, _ in lay.groups:
            dw[:, off // 16:(off + L) // 16] = _wrap16(
                all_didx[c][off:off + L].astype(np.int16))
        in_maps.append({
            "eaT": eaT, "xT": xT, "srel": all_srel[c].reshape(1, ec),
            "didx": dw, "iota": iota,
            "wlin": wlin_h, "w1": w1_h, "w2": w2_h, "blin": blin_h,
            "g1": g1_h, "be1": be1_h, "g2": g2_h, "be2": be2_h,
        })
    return lay, in_maps, all_inv


_GRAPH_CACHE = {}


def get_graph(lay: Layout):
    if lay.key not in _GRAPH_CACHE:
        _GRAPH_CACHE[lay.key] = build_graph(lay)
    return _GRAPH_CACHE[lay.key]


def kernel(x, edge_index, edge_attr, W_lin, b_lin, W1, b1, g1, be1, W2, b2,
           g2, be2):
    """Full-input entry point: shard, run on 8 NeuronCores, gather."""
    x = np.asarray(x)
    edge_index = np.asarray(edge_index)
    edge_attr = np.asarray(edge_attr)
    e = edge_attr.shape[0]
    ec0 = e // N_CORES

    lay, in_maps, invs = make_in_maps(
        x, edge_index, edge_attr, np.asarray(W_lin), np.asarray(b_lin),
        np.asarray(W1), np.asarray(g1), np.asarray(be1), np.asarray(W2),
        np.asarray(g2), np.asarray(be2), N_CORES)
    nc = get_graph(lay)
    res = run_bass_kernel_spmd(nc, in_maps, core_ids=list(range(N_CORES)))
    out = np.empty((e, NIN), dtype=np.float32)
    for c in range(N_CORES):
        oT = np.asarray(res.results[c]["outT"], dtype=np.float32)
        out[c * ec0:(c + 1) * ec0] = oT.T[invs[c]]
    return out


# revision 20
# speedup vs baseline: 1.0098x; 1.0036x over previous
"""Trainium2 Bass kernel for the GNN edge-update MLP (8 NeuronCores).

Reference semantics:
    h   = x @ W_lin.T + b_lin                       # [N, nin]
    agg = h[src] + h[dst]                           # [E, nin]
    z   = concat([agg, edge_attr], -1)              # [E, 2*nin]
    z   = relu(BN(z @ W1.T + b1; g1, be1))          # [E, nout]  (BN over edges)
    z   = relu(BN(z @ W2.T + b2; g2, be2))          # [E, nout]

Restructuring (v2 — one-hot src path):
  * b1/b2 cancel inside training-mode BN -> dropped.
  * z @ W1.T = hW[src] + hW[dst] + ea @ W1b.T, with W1 = [W1a | W1b] and
    hW = x @ (W1a W_lin).T + W1a b_lin  (a [N, nout] row-major DRAM table).
  * Edges are sharded over 8 cores; per core they are bucketed by
    (dst >= SPLIT) [int16 gather trick] and SORTED BY SRC inside each bucket.
  * dst contribution: GPSIMD dma_gather from the table (per-edge descriptors,
    ~8ns each — this is the only per-edge SWDGE cost left).
  * src contribution: because edges are src-sorted, each 512-column chunk's
    srcs span only ~2-4 aligned 128-node windows.  For each window the PE
    multiplies the table slice (lhsT [128 nodes, 128 feat]) by a one-hot
    selection matrix S (built on DVE by comparing a broadcast src-value row
    against an iota column) and accumulates into the chunk's PSUM bank.
    No per-edge descriptors at all.
  * The per-chunk window structure (and bucket caps) are data-dependent and
    baked into the compiled graph (compile happens per input set; cached).
  * BN statistics: per-chunk vector bn_stats, merged, AllReduce'd ([128,2]).
    A dummy AllReduce at t=0 absorbs the first-collective setup cost.
  * Padded columns are exactly zero through u1 (src matches no window row,
    dst gathers a zero row, ea is zero); their constant effect on the second
    BN is subtracted analytically (v2 correction), as in v1.
"""

import sys
from contextlib import ExitStack

import numpy as np

try:
    import concourse  # noqa: F401
except ImportError:  # pragma: no cover
    sys.path.insert(0, "/opt/trn_rl_repo")

import ml_dtypes
from concourse import bass, bacc, mybir
from concourse import tile
from concourse.bass_utils import run_bass_kernel_spmd
from concourse.masks import make_identity

BF16 = ml_dtypes.bfloat16
FP16 = np.float16

N_CORES = 8
NIN = 128
EPS = 1e-5
P = 128

SPLIT = 32640            # nodes < SPLIT are "lo", >= SPLIT are "hi" (128-mult)
GROUP = 2048             # edges per dma_gather instruction
CHUNK = 512
RUN = 512                # table rows per window-run load (4 windows)
MAXW = 16                # max windows (of 128 rows) spanned by one chunk


def _r512(v):
    return ((v + 511) // 512) * 512


def table_layout(n_nodes):
    """One DRAM table, hi region first (rows [0, hi_rows)), then lo."""
    nhi = n_nodes - SPLIT
    hi_rows = _r512(nhi + 1)
    lo_rows = _r512(SPLIT + 1)
    return nhi, hi_rows, lo_rows, hi_rows + lo_rows


class Layout:
    """Per-compile structural data (hashable via .key)."""

    def __init__(self, caps, chunks, groups, runs, n_nodes, e_total):
        self.caps = caps          # (cap_hi, cap_lo) in BUCKET order (hi, lo)
        self.chunks = chunks      # [(off, gi, [(run_i, k, dw), ...])]
        self.groups = groups      # [(off, L, dst_hi)]
        self.runs = runs          # [table_row0] per run (RUN rows each)
        self.n_nodes = n_nodes
        self.e_total = e_total
        self.ec = sum(caps)
        self.key = (caps, tuple((o, g, tuple(w)) for o, g, w in chunks),
                    tuple(groups), tuple(runs), n_nodes, e_total)


def build_graph(lay: Layout, n_cores=N_CORES, eps=EPS):
    f32 = mybir.dt.float32
    bf16 = mybir.dt.bfloat16
    fp16 = mybir.dt.float16
    f8 = mybir.dt.float8e4
    i16 = mybir.dt.int16
    FT = mybir.ActivationFunctionType

    nc = bacc.Bacc(
        "TRN2", target_bir_lowering=False, debug=False, num_devices=n_cores
    )

    nhi, hi_rows, lo_rows, npad = table_layout(lay.n_nodes)
    ec = lay.ec
    e_total = lay.e_total
    n_pad_tot = ec * n_cores - e_total
    nchunk = ec // CHUNK
    nstat = nchunk

    # ---- I/O -------------------------------------------------------------
    eaT = nc.dram_tensor("eaT", [P, ec], bf16, kind="ExternalInput").ap()
    xT = nc.dram_tensor("xT", [P, npad], bf16, kind="ExternalInput").ap()
    srel = nc.dram_tensor("srel", [1, ec], fp16, kind="ExternalInput").ap()
    didx = nc.dram_tensor("didx", [P, ec // 16], i16, kind="ExternalInput").ap()
    iota = nc.dram_tensor("iota", [P, MAXW], f32, kind="ExternalInput").ap()
    wlin = nc.dram_tensor("wlin", [P, P], f32, kind="ExternalInput").ap()
    w1 = nc.dram_tensor("w1", [P, 2 * P], f32, kind="ExternalInput").ap()
    w2 = nc.dram_tensor("w2", [P, P], f32, kind="ExternalInput").ap()
    blin = nc.dram_tensor("blin", [P, 1], f32, kind="ExternalInput").ap()
    g1 = nc.dram_tensor("g1", [P, 1], f32, kind="ExternalInput").ap()
    be1 = nc.dram_tensor("be1", [P, 1], f32, kind="ExternalInput").ap()
    g2 = nc.dram_tensor("g2", [P, 1], f32, kind="ExternalInput").ap()
    be2 = nc.dram_tensor("be2", [P, 1], f32, kind="ExternalInput").ap()
    outT = nc.dram_tensor("outT", [P, ec], bf16, kind="ExternalOutput").ap()

    table = nc.dram_tensor("hw_table", [npad, P], bf16).ap()

    grp_all = [list(range(n_cores))]

    with tile.TileContext(nc) as tc, ExitStack() as es:
        consts = es.enter_context(tc.tile_pool(name="consts", bufs=1))
        gidx = es.enter_context(tc.tile_pool(name="gidx", bufs=4))
        dram = es.enter_context(tc.tile_pool(name="dram", bufs=1, space="DRAM"))
        big = es.enter_context(tc.tile_pool(name="big", bufs=1))
        red = es.enter_context(tc.tile_pool(name="red", bufs=1))

        # ---- warm-up collective (absorbs first-cc setup latency) --------
        wu_in = dram.tile([P, 2], f32, tag="wu_in")
        wu_sb = red.tile([P, 2], f32, tag="wu_sb")
        nc.vector.memset(wu_sb[:], 0.0)
        nc.sync.dma_start(out=wu_in[:], in_=wu_sb[:])
        wu_out = dram.tile([P, 2], f32, tag="wu_out")
        nc.gpsimd.collective_compute(
            "AllReduce", mybir.AluOpType.add, replica_groups=grp_all,
            ins=[wu_in[:].opt()], outs=[wu_out[:].opt()])

        # ---- constants / weight prep ------------------------------------
        ident_f = consts.tile([P, P], f32)
        make_identity(nc, ident_f[:])

        wlin_s = consts.tile([P, P], f32)
        nc.sync.dma_start(out=wlin_s[:], in_=wlin)
        w1_s = consts.tile([P, 2 * P], f32)
        nc.sync.dma_start(out=w1_s[:], in_=w1)
        w2_s = consts.tile([P, P], f32)
        nc.sync.dma_start(out=w2_s[:], in_=w2)
        blin_s = consts.tile([P, 1], f32)
        nc.sync.dma_start(out=blin_s[:], in_=blin)
        g1_s = consts.tile([P, 1], f32)
        nc.sync.dma_start(out=g1_s[:], in_=g1)
        be1_s = consts.tile([P, 1], f32)
        nc.sync.dma_start(out=be1_s[:], in_=be1)
        g2_s = consts.tile([P, 1], f32)
        nc.sync.dma_start(out=g2_s[:], in_=g2)
        be2_s = consts.tile([P, 1], f32)
        nc.sync.dma_start(out=be2_s[:], in_=be2)
        iota_s = consts.tile([P, MAXW], f32)
        nc.sync.dma_start(out=iota_s[:], in_=iota)
        eps_s = consts.tile([P, 1], f32)
        nc.vector.memset(eps_s[:], eps)
        ones16 = consts.tile([1, P], fp16)
        nc.vector.memset(ones16[:], 1.0)

        # prefetch dst idx for the first groups while the table builds
        idx_pre = {}
        for gi, (off, L, _dh) in enumerate(lay.groups[:4]):
            di = gidx.tile([P, GROUP // 16], i16, tag="di")
            nc.sync.dma_start(out=di[:, :L // 16],
                              in_=didx[:, off // 16:(off + L) // 16])
            idx_pre[gi] = di

        w1aT = consts.tile([P, P], f32)
        w1bT = consts.tile([P, P], bf16)
        w2T = consts.tile([P, P], bf16)
        wcT = consts.tile([P, P], bf16)
        bc = consts.tile([P, 1], f32)

        with tc.tile_pool(name="psum0", bufs=1, space="PSUM") as psw, \
             tc.tile_pool(name="psum0b", bufs=3, space="PSUM") as ps0:
            pw = psw.tile([P, P], f32, tag="pw")
            nc.tensor.matmul(pw[:], lhsT=w1_s[:, 0:P], rhs=ident_f[:],
                             start=True, stop=True)
            nc.vector.tensor_copy(w1aT[:], pw[:])
            pw = psw.tile([P, P], f32, tag="pw")
            nc.tensor.matmul(pw[:], lhsT=w1_s[:, P:2 * P], rhs=ident_f[:],
                             start=True, stop=True)
            nc.vector.tensor_copy(w1bT[:], pw[:])
            pw = psw.tile([P, P], f32, tag="pw")
            nc.tensor.matmul(pw[:], lhsT=w2_s[:], rhs=ident_f[:],
                             start=True, stop=True)
            nc.vector.tensor_copy(w2T[:], pw[:])
            # WcT[i, o] = (W1a @ W_lin)[o, i]
            pw = psw.tile([P, P], f32, tag="pw")
            nc.tensor.matmul(pw[:], lhsT=wlin_s[:], rhs=w1aT[:],
                             start=True, stop=True)
            nc.vector.tensor_copy(wcT[:], pw[:])
            pb = psw.tile([P, 1], f32, tag="pb")
            nc.tensor.matmul(pb[:], lhsT=w1aT[:], rhs=blin_s[:],
                             start=True, stop=True)
            nc.vector.tensor_copy(bc[:], pb[:])

            ident_b = consts.tile([P, P], bf16)
            nc.vector.tensor_copy(ident_b[:], ident_f[:])

            # ---- phase 0: build the hW table (hi region first) ----------
            with tc.tile_pool(name="ph0", bufs=4) as ph0:
                zrow = ph0.tile([P, P], bf16, tag="zrow")
                nc.vector.memset(zrow[:], 0.0)

                def build(row0, xcol0, nch):
                    for j in range(nch):
                        xt = ph0.tile([P, 512], bf16, tag="xt")
                        nc.sync.dma_start(
                            out=xt[:],
                            in_=xT[:, xcol0 + j * 512:xcol0 + (j + 1) * 512])
                        hp = ps0.tile([P, 512], f32, tag="hp")
                        nc.tensor.matmul(hp[:], lhsT=wcT[:], rhs=xt[:],
                                         start=True, stop=True)
                        hs = ph0.tile([P, 512], bf16, tag="hs")
                        nc.scalar.activation(hs[:], hp[:], func=FT.Identity,
                                             bias=bc[:], scale=1.0)
                        tp = ps0.tile([P, 512], f32, tag="tp")
                        for s in range(4):
                            nc.tensor.matmul(tp[:, s * P:(s + 1) * P],
                                             lhsT=hs[:, s * P:(s + 1) * P],
                                             rhs=ident_b[:], start=True,
                                             stop=True)
                        ts = ph0.tile([P, 512], bf16, tag="ts")
                        nc.vector.tensor_copy(ts[:], tp[:])
                        r = row0 + j * 512
                        nc.sync.dma_start(
                            out=table[r:r + 512, :].rearrange(
                                "(s p) o -> p s o", p=P),
                            in_=ts[:].rearrange("p (s o) -> p s o", s=4),
                        )

                build(0, 0, hi_rows // 512)
                nc.sync.dma_start(out=table[nhi:nhi + 1, :], in_=zrow[0:1, :])
                build(hi_rows, hi_rows, lo_rows // 512)
                zlo = hi_rows + SPLIT
                nc.sync.dma_start(out=table[zlo:zlo + 1, :], in_=zrow[0:1, :])

        u1 = big.tile([P, ec], bf16)
        stats = consts.tile([P, nstat, 6], f32)

        def bn_coeffs(g_s, be_s, nck, corr=None):
            se = red.tile([P, nstat], f32, tag="se")
            nc.vector.tensor_mul(se[:, :nck], stats[:, :nck, 0],
                                 stats[:, :nck, 1])
            so = red.tile([P, nstat], f32, tag="so")
            nc.vector.tensor_mul(so[:, :nck], stats[:, :nck, 3],
                                 stats[:, :nck, 4])
            qe = red.tile([P, nstat], f32, tag="qe")
            nc.vector.tensor_mul(qe[:, :nck], se[:, :nck], stats[:, :nck, 1])
            nc.vector.tensor_add(qe[:, :nck], qe[:, :nck], stats[:, :nck, 2])
            qo = red.tile([P, nstat], f32, tag="qo")
            nc.vector.tensor_mul(qo[:, :nck], so[:, :nck], stats[:, :nck, 4])
            nc.vector.tensor_add(qo[:, :nck], qo[:, :nck], stats[:, :nck, 5])
            nc.vector.tensor_add(se[:, :nck], se[:, :nck], so[:, :nck])
            nc.vector.tensor_add(qe[:, :nck], qe[:, :nck], qo[:, :nck])
            sq = red.tile([P, 2], f32, tag="sq")
            nc.vector.tensor_reduce(sq[:, 0:1], se[:, :nck],
                                    axis=mybir.AxisListType.X,
                                    op=mybir.AluOpType.add)
            nc.vector.tensor_reduce(sq[:, 1:2], qe[:, :nck],
                                    axis=mybir.AxisListType.X,
                                    op=mybir.AluOpType.add)
            cc_in = dram.tile([P, 2], f32, tag="cc_in")
            nc.sync.dma_start(out=cc_in[:], in_=sq[:])
            cc_out = dram.tile([P, 2], f32, tag="cc_out")
            nc.gpsimd.collective_compute(
                "AllReduce", mybir.AluOpType.add, replica_groups=grp_all,
                ins=[cc_in[:].opt()], outs=[cc_out[:].opt()])
            sqg = red.tile([P, 2], f32, tag="sqg")
            nc.sync.dma_start(out=sqg[:], in_=cc_out[:])
            if corr is not None:
                v, vq = corr
                t = red.tile([P, 2], f32, tag="tcorr")
                nc.vector.tensor_scalar_mul(t[:, 0:1], v[:], float(n_pad_tot))
                nc.vector.tensor_scalar_mul(t[:, 1:2], vq[:], float(n_pad_tot))
                nc.vector.tensor_sub(sqg[:], sqg[:], t[:])
            mu = red.tile([P, 1], f32, tag="mu")
            nc.vector.tensor_scalar_mul(mu[:], sqg[:, 0:1], 1.0 / e_total)
            var = red.tile([P, 1], f32, tag="var")
            nc.vector.tensor_scalar_mul(var[:], sqg[:, 1:2], 1.0 / e_total)
            mu2 = red.tile([P, 1], f32, tag="mu2")
            nc.vector.tensor_mul(mu2[:], mu[:], mu[:])
            nc.vector.tensor_sub(var[:], var[:], mu2[:])
            a = red.tile([P, 1], f32, tag="a")
            nc.scalar.activation(a[:], var[:], func=FT.Sqrt, bias=eps_s[:],
                                 scale=1.0)
            nc.vector.reciprocal(a[:], a[:])
            nc.vector.tensor_mul(a[:], a[:], g_s[:])
            c = red.tile([P, 1], f32, tag="c")
            nc.vector.tensor_mul(c[:], mu[:], a[:])
            nc.vector.tensor_sub(c[:], be_s[:], c[:])
            return a, c

        with (
            tc.tile_pool(name="psA", bufs=4, space="PSUM") as psA,
            tc.tile_pool(name="psB", bufs=2, space="PSUM") as psB,
            tc.tile_pool(name="psS", bufs=1, space="PSUM") as psS,
            tc.tile_pool(name="ea", bufs=2) as eap,
            tc.tile_pool(name="sr", bufs=3) as srp,
            tc.tile_pool(name="gp", bufs=3) as gp,
            tc.tile_pool(name="wr", bufs=3) as wrp,
            tc.tile_pool(name="sp", bufs=3) as Sp,
            tc.tile_pool(name="op", bufs=2) as op,
        ):
            # ---- pass A ------------------------------------------------
            # dst gathers, one per group (queued up front; gp bufs throttle)
            g_tiles = {}
            for gi, (off, L, dst_hi) in enumerate(lay.groups):
                if gi in idx_pre:
                    di = idx_pre[gi]
                else:
                    di = gidx.tile([P, GROUP // 16], i16, tag="di")
                    nc.sync.dma_start(out=di[:, :L // 16],
                                      in_=didx[:, off // 16:(off + L) // 16])
                gdst = gp.tile([P, GROUP], bf16, tag="gdst")
                base = table[0:hi_rows, :] if dst_hi \
                    else table[hi_rows:npad, :]
                nc.gpsimd.dma_gather(
                    out_ap=gdst[:, :L].rearrange("p (a s) -> p a s", a=1),
                    in_ap=base, idxs_ap=di[:, :L // 16],
                    num_idxs=L, num_idxs_reg=L, elem_size=P,
                    transpose=True, single_packet=False)
                g_tiles[gi] = (gdst, off)

            # group-level ea staging
            ea_tiles = {}

            run_tiles = {}
            caps_hi = lay.caps[0]

            for ci, (off, gi, wins) in enumerate(lay.chunks):
                bkt = 0 if off < caps_hi else 1
                goff, gL, _ = lay.groups[gi]
                if gi not in ea_tiles:
                    et = eap.tile([P, GROUP], bf16, tag="ea")
                    nc.sync.dma_start(out=et[:, :gL],
                                      in_=eaT[:, goff:goff + gL])
                    ea_tiles = {gi: et}
                et = ea_tiles[gi]
                rel = off - goff

                up = psA.tile([P, CHUNK], f32, tag="up")
                nc.tensor.matmul(up[:], lhsT=w1bT[:], rhs=et[:, rel:rel + CHUNK],
                                 start=True, stop=(len(wins) == 0),
                                 skip_group_check=bool(wins))

                if wins:
                    clo = min(w[3] for w in wins)
                    chi = max(w[4] for w in wins)
                    st = srp.tile([1, CHUNK], fp16, tag="sr")
                    nc.scalar.dma_start(out=st[:, clo:chi],
                                        in_=srel[:, off + clo:off + chi])
                    bps = psB.tile([P, CHUNK], f32, tag="bps")
                    nc.tensor.matmul(bps[:, clo:chi], lhsT=ones16[:],
                                     rhs=st[:, clo:chi],
                                     start=True, stop=True)
                    for wi, (run_i, k, dw, a, b) in enumerate(wins):
                        rkey = (bkt, run_i)
                        if rkey not in run_tiles:
                            wt = wrp.tile([P, RUN], bf16, tag="wt")
                            r0 = lay.runs[run_i]
                            nc.scalar.dma_start(
                                out=wt[:].rearrange("p (k f) -> p k f",
                                                    k=RUN // P),
                                in_=table[r0:r0 + RUN, :].rearrange(
                                    "(k p) f -> p k f", p=P))
                            run_tiles[rkey] = wt
                        wt = run_tiles[rkey]
                        S_w = Sp.tile([P, CHUNK], bf16, tag="S")
                        nc.vector.tensor_scalar(
                            out=S_w[:, a:b], in0=bps[:, a:b],
                            scalar1=iota_s[:, dw:dw + 1], scalar2=None,
                            op0=mybir.AluOpType.is_equal)
                        nc.tensor.matmul(up[:, a:b],
                                         lhsT=wt[:, k * P:(k + 1) * P],
                                         rhs=S_w[:, a:b], start=False,
                                         stop=(wi == len(wins) - 1),
                                         skip_group_check=True)

                gdst, g_off = g_tiles[gi]
                grel = off - g_off
                nc.vector.tensor_add(u1[:, off:off + CHUNK], up[:],
                                     gdst[:, grel:grel + CHUNK])
                nc.vector.bn_stats(stats[:, ci, :], u1[:, off:off + CHUNK])

            a1, c1 = bn_coeffs(g1_s, be1_s, nchunk)

            # pad columns have u1 == 0 -> u2_pad = W2 @ relu(c1), constant
            rc = red.tile([P, 1], f32, tag="rc")
            nc.scalar.activation(rc[:], c1[:], func=FT.Relu)
            rcb = red.tile([P, 1], bf16, tag="rcb")
            nc.vector.tensor_copy(rcb[:], rc[:])
            vp = psS.tile([P, 1], f32, tag="vp")
            nc.tensor.matmul(vp[:], lhsT=w2T[:], rhs=rcb[:],
                             start=True, stop=True)
            v2 = red.tile([P, 1], f32, tag="v2")
            nc.vector.tensor_copy(v2[:], vp[:])
            v2q = red.tile([P, 1], f32, tag="v2q")
            nc.vector.tensor_mul(v2q[:], v2[:], v2[:])

            # ---- pass B: z1 = relu(a1*u1+c1) in place; stats of W2@z1 ---
            for k in range(nchunk):
                off = k * CHUNK
                nc.scalar.activation(u1[:, off:off + CHUNK],
                                     u1[:, off:off + CHUNK],
                                     func=FT.Relu, scale=a1[:], bias=c1[:])
                up = psA.tile([P, CHUNK], f32, tag="up")
                nc.tensor.matmul(up[:], lhsT=w2T[:],
                                 rhs=u1[:, off:off + CHUNK],
                                 start=True, stop=True)
                nc.vector.bn_stats(stats[:, k, :], up[:])

            a2, c2 = bn_coeffs(g2_s, be2_s, nchunk, corr=(v2, v2q))

            # ---- pass C: out = relu(a2*(W2@z1)+c2), staged per group ----
            for base in range(0, ec, GROUP):
                gL = min(GROUP, ec - base)
                ot = op.tile([P, GROUP], bf16, tag="ot")
                for off in range(base, base + gL, CHUNK):
                    up = psA.tile([P, CHUNK], f32, tag="up")
                    nc.tensor.matmul(up[:], lhsT=w2T[:],
                                     rhs=u1[:, off:off + CHUNK],
                                     start=True, stop=True)
                    r = off - base
                    nc.scalar.activation(ot[:, r:r + CHUNK], up[:],
                                         func=FT.Relu, scale=a2[:],
                                         bias=c2[:])
                nc.sync.dma_start(out=outT[:, base:base + gL],
                                  in_=ot[:, :gL])

    nc.compile()
    return nc


def _wrap16(a):
    """linear [L] -> [16, L/16] wrapped, tiled to [128, L/16]."""
    w = np.ascontiguousarray(a.reshape(-1, 16).T)
    return np.tile(w, (8, 1))


def host_prep(x, edge_index, edge_attr, n_cores):
    """Shard, bucket by dst-region, sort by src, build layout + per-core
    arrays."""
    n = x.shape[0]
    e = edge_attr.shape[0]
    ec0 = e // n_cores
    nhi, hi_rows, lo_rows, npad = table_layout(n)

    src_all = edge_index[0].astype(np.int64)
    dst_all = edge_index[1].astype(np.int64)

    per_core = []
    counts = np.zeros((n_cores, 2), np.int64)
    for c in range(n_cores):
        sl = slice(c * ec0, (c + 1) * ec0)
        s, d = src_all[sl], dst_all[sl]
        hi = (d >= SPLIT).astype(np.int64)
        order = np.argsort(hi * (1 << 32) + s, kind="stable")
        counts[c, 1] = int(hi.sum())          # bucket 1 = hi
        counts[c, 0] = ec0 - counts[c, 1]
        per_core.append((s, d, hi, order))

    # bucket order: hi first (table hi region builds first), caps %512
    caps = (int(_r512(counts[:, 1].max())), int(_r512(counts[:, 0].max())))
    ec = caps[0] + caps[1]
    bucket_off = {1: 0, 0: caps[0]}

    zero_hi = nhi                 # local idx of zero row in hi region
    zero_lo = SPLIT               # local idx of zero row in lo region

    # groups (shared across cores: same caps)
    groups = []
    for b, cap in ((1, caps[0]), (0, caps[1])):
        off = bucket_off[b]
        rem = cap
        while rem > 0:
            L = min(GROUP, rem)
            groups.append((off, L, b == 1))
            off += L
            rem -= L

    # per-core padded arrays + union chunk windows
    all_srel = []
    all_didx = []
    all_eacols = []
    all_inv = []
    win_ranges = [dict() for _ in range(ec // CHUNK)]
    w0_arr = np.zeros(ec // CHUNK, np.int64)

    # first pass: compute padded src arrays to derive union windows
    src_p_all = []
    for c in range(n_cores):
        s, d, hi, order = per_core[c]
        src_p = np.full(ec, -1, np.int64)
        dst_p = np.empty(ec, np.int64)
        ea_cols = np.full(ec, -1, np.int64)
        # order sorts by (hi, src): lo bucket first in order, but hi bucket
        # comes first in columns.
        n_lo = int(counts[c, 0])
        idx_lo = order[:n_lo]
        idx_hi = order[n_lo:]
        for b, idx_b in ((1, idx_hi), (0, idx_lo)):
            cnt = len(idx_b)
            off = bucket_off[b]
            pos = off + np.arange(cnt)
            src_p[pos] = s[idx_b]
            dst_p[pos] = d[idx_b] - (SPLIT if b == 1 else 0)
            ea_cols[pos] = idx_b
            padr = np.arange(off + cnt, off + (caps[0] if b == 1 else caps[1]))
            dst_p[padr] = zero_hi if b == 1 else zero_lo
        inv = np.empty(ec0, np.int64)
        inv[idx_hi] = bucket_off[1] + np.arange(len(idx_hi))
        inv[idx_lo] = bucket_off[0] + np.arange(len(idx_lo))
        src_p_all.append(src_p)
        all_didx.append(dst_p)
        all_eacols.append(ea_cols)
        all_inv.append(inv)

        for ci in range(ec // CHUNK):
            seg = src_p[ci * CHUNK:(ci + 1) * CHUNK]
            for w in np.unique(seg[seg >= 0] // P):
                pos = np.nonzero(seg // P == w)[0]
                a, b = int(pos[0]), int(pos[-1]) + 1
                cur = win_ranges[ci].get(int(w))
                if cur is None:
                    win_ranges[ci][int(w)] = [a, b]
                else:
                    cur[0] = min(cur[0], a)
                    cur[1] = max(cur[1], b)

    # per-chunk base window + srel arrays (shared w0 across cores)
    chunks = []
    run_index = {}
    runs = []
    gi_of_off = {off: i for i, (off, L, _) in enumerate(groups)
                 for off in range(off, off + L, CHUNK)}
    for ci in range(ec // CHUNK):
        wins = sorted(win_ranges[ci])
        off = ci * CHUNK
        wl = []
        if wins:
            w0 = wins[0]
            w0_arr[ci] = w0
            assert wins[-1] - w0 < MAXW, f"chunk {ci} spans {wins}"
            for w in wins:
                node0 = w * P
                if node0 >= SPLIT:
                    row0 = node0 - SPLIT          # hi region
                else:
                    row0 = hi_rows + node0        # lo region
                run0 = (row0 // RUN) * RUN
                if run0 not in run_index:
                    run_index[run0] = len(runs)
                    runs.append(run0)
                a, b = win_ranges[ci][w]
                wl.append((run_index[run0], (row0 - run0) // P, w - w0,
                           a, b))
        chunks.append((off, gi_of_off[off], wl))

    for c in range(n_cores):
        src_p = src_p_all[c]
        srel = np.full(ec, -1.0, np.float32)
        for ci in range(ec // CHUNK):
            seg = src_p[ci * CHUNK:(ci + 1) * CHUNK]
            m = seg >= 0
            srel[ci * CHUNK:(ci + 1) * CHUNK][m] = seg[m] - w0_arr[ci] * P
        assert srel.max() < 2048
        all_srel.append(srel.astype(FP16))

    lay = Layout(caps, chunks, groups, runs, n, e)
    return lay, all_srel, all_didx, all_eacols, all_inv


def make_in_maps(x, edge_index, edge_attr, W_lin, b_lin, W1, g1, be1, W2,
                 g2, be2, n_cores):
    n = x.shape[0]
    nhi, hi_rows, lo_rows, npad = table_layout(n)
    lay, all_srel, all_didx, all_eacols, all_inv = host_prep(
        x, edge_index, edge_attr, n_cores)
    ec = lay.ec
    ec0 = edge_attr.shape[0] // n_cores

    xbf = x.astype(BF16)
    xT = np.zeros((P, npad), dtype=BF16)
    xT[:, 0:nhi] = xbf[SPLIT:n].T
    xT[:, hi_rows:hi_rows + SPLIT] = xbf[0:SPLIT].T

    iota = (np.arange(P)[:, None]
            + P * np.arange(MAXW)[None, :]).astype(np.float32)

    f32c = np.ascontiguousarray
    wlin_h = f32c(W_lin.astype(np.float32))
    w1_h = f32c(W1.astype(np.float32))
    w2_h = f32c(W2.astype(np.float32))
    blin_h = f32c(b_lin.astype(np.float32).reshape(P, 1))
    g1_h = f32c(g1.astype(np.float32).reshape(P, 1))
    be1_h = f32c(be1.astype(np.float32).reshape(P, 1))
    g2_h = f32c(g2.astype(np.float32).reshape(P, 1))
    be2_h = f32c(be2.astype(np.float32).reshape(P, 1))

    eabf = edge_attr.astype(BF16)

    in_maps = []
    for c in range(n_cores):
        ea_cols = all_eacols[c]
        eaT = np.zeros((P, ec), dtype=BF16)
        real = ea_cols >= 0
        eaT[:, real] = eabf[c * ec0 + ea_cols[real]].T
        dw = np.zeros((P, ec // 16), np.int16)
        for off, L, _ in lay.groups:
            dw[:, off // 16:(off + L) // 16] = _wrap16(
                all_didx[c][off:off + L].astype(np.int16))
        in_maps.append({
            "eaT": eaT, "xT": xT, "srel": all_srel[c].reshape(1, ec),
            "didx": dw, "iota": iota,
            "wlin": wlin_h, "w1": w1_h, "w2": w2_h, "blin": blin_h,
            "g1": g1_h, "be1": be1_h, "g2": g2_h, "be2": be2_h,
        })
    return lay, in_maps, all_inv


_GRAPH_CACHE = {}


def get_graph(lay: Layout):
    if lay.key not in _GRAPH_CACHE:
        _GRAPH_CACHE[lay.key] = build_graph(lay)
    return _GRAPH_CACHE[lay.key]


def kernel(x, edge_index, edge_attr, W_lin, b_lin, W1, b1, g1, be1, W2, b2,
           g2, be2):
    """Full-input entry point: shard, run on 8 NeuronCores, gather."""
    x = np.asarray(x)
    edge_index = np.asarray(edge_index)
    edge_attr = np.asarray(edge_attr)
    e = edge_attr.shape[0]
    ec0 = e // N_CORES

    lay, in_maps, invs = make_in_maps(
        x, edge_index, edge_attr, np.asarray(W_lin), np.asarray(b_lin),
        np.asarray(W1), np.asarray(g1), np.asarray(be1), np.asarray(W2),
        np.asarray(g2), np.asarray(be2), N_CORES)
    nc = get_graph(lay)
    res = run_bass_kernel_spmd(nc, in_maps, core_ids=list(range(N_CORES)))
    out = np.empty((e, NIN), dtype=np.float32)
    for c in range(N_CORES):
        oT = np.asarray(res.results[c]["outT"], dtype=np.float32)
        out[c * ec0:(c + 1) * ec0] = oT.T[invs[c]]
    return out


# revision 21
# speedup vs baseline: 1.0486x; 1.0385x over previous
"""Trainium2 Bass kernel for the GNN edge-update MLP (8 NeuronCores).

Reference semantics:
    h   = x @ W_lin.T + b_lin                       # [N, nin]
    agg = h[src] + h[dst]                           # [E, nin]
    z   = concat([agg, edge_attr], -1)              # [E, 2*nin]
    z   = relu(BN(z @ W1.T + b1; g1, be1))          # [E, nout]  (BN over edges)
    z   = relu(BN(z @ W2.T + b2; g2, be2))          # [E, nout]

Restructuring (v2 — one-hot src path):
  * b1/b2 cancel inside training-mode BN -> dropped.
  * z @ W1.T = hW[src] + hW[dst] + ea @ W1b.T, with W1 = [W1a | W1b] and
    hW = x @ (W1a W_lin).T + W1a b_lin  (a [N, nout] row-major DRAM table).
  * Edges are sharded over 8 cores; per core they are bucketed by
    (dst >= SPLIT) [int16 gather trick] and SORTED BY SRC inside each bucket.
  * dst contribution: GPSIMD dma_gather from the table (per-edge descriptors,
    ~8ns each — this is the only per-edge SWDGE cost left).
  * src contribution: because edges are src-sorted, each 512-column chunk's
    srcs span only ~2-4 aligned 128-node windows.  For each window the PE
    multiplies the table slice (lhsT [128 nodes, 128 feat]) by a one-hot
    selection matrix S (built on DVE by comparing a broadcast src-value row
    against an iota column) and accumulates into the chunk's PSUM bank.
    No per-edge descriptors at all.
  * The per-chunk window structure (and bucket caps) are data-dependent and
    baked into the compiled graph (compile happens per input set; cached).
  * BN statistics: per-chunk vector bn_stats, merged, AllReduce'd ([128,2]).
    A dummy AllReduce at t=0 absorbs the first-collective setup cost.
  * Padded columns are exactly zero through u1 (src matches no window row,
    dst gathers a zero row, ea is zero); their constant effect on the second
    BN is subtracted analytically (v2 correction), as in v1.
"""

import sys
from contextlib import ExitStack

import numpy as np

try:
    import concourse  # noqa: F401
except ImportError:  # pragma: no cover
    sys.path.insert(0, "/opt/trn_rl_repo")

import ml_dtypes
from concourse import bass, bacc, mybir
from concourse import tile
from concourse.bass_utils import run_bass_kernel_spmd
from concourse.masks import make_identity

BF16 = ml_dtypes.bfloat16
FP16 = np.float16

N_CORES = 8
NIN = 128
EPS = 1e-5
P = 128

SPLIT = 32640            # nodes < SPLIT are "lo", >= SPLIT are "hi" (128-mult)
GROUP = 2048             # edges per dma_gather instruction
CHUNK = 512
RUN = 512                # table rows per window-run load (4 windows)
MAXW = 16                # max windows (of 128 rows) spanned by one chunk


def _r512(v):
    return ((v + 511) // 512) * 512


def table_layout(n_nodes):
    """One DRAM table, hi region first (rows [0, hi_rows)), then lo."""
    nhi = n_nodes - SPLIT
    hi_rows = _r512(nhi + 1)
    lo_rows = _r512(SPLIT + 1)
    return nhi, hi_rows, lo_rows, hi_rows + lo_rows


class Layout:
    """Per-compile structural data (hashable via .key)."""

    def __init__(self, caps, chunks, groups, runs, n_nodes, e_total):
        self.caps = caps          # (cap_hi, cap_lo) in BUCKET order (hi, lo)
        self.chunks = chunks      # [(off, gi, [(run_i, k, dw), ...])]
        self.groups = groups      # [(off, L, dst_hi)]
        self.runs = runs          # [table_row0] per run (RUN rows each)
        self.n_nodes = n_nodes
        self.e_total = e_total
        self.ec = sum(caps)
        self.key = (caps, tuple((o, g, tuple(w)) for o, g, w in chunks),
                    tuple(groups), tuple(runs), n_nodes, e_total)


def build_graph(lay: Layout, n_cores=N_CORES, eps=EPS):
    f32 = mybir.dt.float32
    bf16 = mybir.dt.bfloat16
    fp16 = mybir.dt.float16
    f8 = mybir.dt.float8e4
    i16 = mybir.dt.int16
    FT = mybir.ActivationFunctionType

    nc = bacc.Bacc(
        "TRN2", target_bir_lowering=False, debug=False, num_devices=n_cores
    )

    nhi, hi_rows, lo_rows, npad = table_layout(lay.n_nodes)
    ec = lay.ec
    e_total = lay.e_total
    n_pad_tot = ec * n_cores - e_total
    nchunk = ec // CHUNK
    nstat = nchunk

    # ---- I/O -------------------------------------------------------------
    eaT = nc.dram_tensor("eaT", [P, ec], bf16, kind="ExternalInput").ap()
    xT = nc.dram_tensor("xT", [P, npad], bf16, kind="ExternalInput").ap()
    srel = nc.dram_tensor("srel", [1, ec], fp16, kind="ExternalInput").ap()
    didx = nc.dram_tensor("didx", [P, ec // 16], i16, kind="ExternalInput").ap()
    iota = nc.dram_tensor("iota", [P, MAXW], f32, kind="ExternalInput").ap()
    wlin = nc.dram_tensor("wlin", [P, P], f32, kind="ExternalInput").ap()
    w1 = nc.dram_tensor("w1", [P, 2 * P], f32, kind="ExternalInput").ap()
    w2 = nc.dram_tensor("w2", [P, P], f32, kind="ExternalInput").ap()
    blin = nc.dram_tensor("blin", [P, 1], f32, kind="ExternalInput").ap()
    g1 = nc.dram_tensor("g1", [P, 1], f32, kind="ExternalInput").ap()
    be1 = nc.dram_tensor("be1", [P, 1], f32, kind="ExternalInput").ap()
    g2 = nc.dram_tensor("g2", [P, 1], f32, kind="ExternalInput").ap()
    be2 = nc.dram_tensor("be2", [P, 1], f32, kind="ExternalInput").ap()
    outT = nc.dram_tensor("outT", [P, ec], bf16, kind="ExternalOutput").ap()

    table_hi = nc.dram_tensor("hw_table_hi", [hi_rows, P], bf16).ap()
    table_lo = nc.dram_tensor("hw_table_lo", [lo_rows, P], bf16).ap()

    grp_all = [list(range(n_cores))]

    with tile.TileContext(nc) as tc, ExitStack() as es:
        consts = es.enter_context(tc.tile_pool(name="consts", bufs=1))
        gidx = es.enter_context(tc.tile_pool(name="gidx", bufs=4))
        dram = es.enter_context(tc.tile_pool(name="dram", bufs=1, space="DRAM"))
        big = es.enter_context(tc.tile_pool(name="big", bufs=1))
        red = es.enter_context(tc.tile_pool(name="red", bufs=1))

        # ---- warm-up collective (absorbs first-cc setup latency) --------
        wu_in = dram.tile([P, 2], f32, tag="wu_in")
        wu_sb = red.tile([P, 2], f32, tag="wu_sb")
        nc.vector.memset(wu_sb[:], 0.0)
        nc.sync.dma_start(out=wu_in[:], in_=wu_sb[:])
        wu_out = dram.tile([P, 2], f32, tag="wu_out")
        nc.gpsimd.collective_compute(
            "AllReduce", mybir.AluOpType.add, replica_groups=grp_all,
            ins=[wu_in[:].opt()], outs=[wu_out[:].opt()])

        # ---- constants / weight prep ------------------------------------
        ident_f = consts.tile([P, P], f32)
        make_identity(nc, ident_f[:])

        wlin_s = consts.tile([P, P], f32)
        nc.sync.dma_start(out=wlin_s[:], in_=wlin)
        w1_s = consts.tile([P, 2 * P], f32)
        nc.sync.dma_start(out=w1_s[:], in_=w1)
        w2_s = consts.tile([P, P], f32)
        nc.sync.dma_start(out=w2_s[:], in_=w2)
        blin_s = consts.tile([P, 1], f32)
        nc.sync.dma_start(out=blin_s[:], in_=blin)
        g1_s = consts.tile([P, 1], f32)
        nc.sync.dma_start(out=g1_s[:], in_=g1)
        be1_s = consts.tile([P, 1], f32)
        nc.sync.dma_start(out=be1_s[:], in_=be1)
        g2_s = consts.tile([P, 1], f32)
        nc.sync.dma_start(out=g2_s[:], in_=g2)
        be2_s = consts.tile([P, 1], f32)
        nc.sync.dma_start(out=be2_s[:], in_=be2)
        iota_s = consts.tile([P, MAXW], f32)
        nc.sync.dma_start(out=iota_s[:], in_=iota)
        eps_s = consts.tile([P, 1], f32)
        nc.vector.memset(eps_s[:], eps)
        ones16 = consts.tile([1, P], fp16)
        nc.vector.memset(ones16[:], 1.0)

        # prefetch dst idx for the first groups while the table builds
        idx_pre = {}
        for gi, (off, L, _dh) in enumerate(lay.groups[:4]):
            di = gidx.tile([P, GROUP // 16], i16, tag="di")
            nc.sync.dma_start(out=di[:, :L // 16],
                              in_=didx[:, off // 16:(off + L) // 16])
            idx_pre[gi] = di

        w1aT = consts.tile([P, P], f32)
        w1bT = consts.tile([P, P], bf16)
        w2T = consts.tile([P, P], bf16)
        wcT = consts.tile([P, P], bf16)
        bc = consts.tile([P, 1], f32)

        with tc.tile_pool(name="psum0", bufs=1, space="PSUM") as psw, \
             tc.tile_pool(name="psum0b", bufs=3, space="PSUM") as ps0:
            pw = psw.tile([P, P], f32, tag="pw")
            nc.tensor.matmul(pw[:], lhsT=w1_s[:, 0:P], rhs=ident_f[:],
                             start=True, stop=True)
            nc.vector.tensor_copy(w1aT[:], pw[:])
            pw = psw.tile([P, P], f32, tag="pw")
            nc.tensor.matmul(pw[:], lhsT=w1_s[:, P:2 * P], rhs=ident_f[:],
                             start=True, stop=True)
            nc.vector.tensor_copy(w1bT[:], pw[:])
            pw = psw.tile([P, P], f32, tag="pw")
            nc.tensor.matmul(pw[:], lhsT=w2_s[:], rhs=ident_f[:],
                             start=True, stop=True)
            nc.vector.tensor_copy(w2T[:], pw[:])
            # WcT[i, o] = (W1a @ W_lin)[o, i]
            pw = psw.tile([P, P], f32, tag="pw")
            nc.tensor.matmul(pw[:], lhsT=wlin_s[:], rhs=w1aT[:],
                             start=True, stop=True)
            nc.vector.tensor_copy(wcT[:], pw[:])

            # ---- phase 0: build the hW table row-major (hi first) -------
            with tc.tile_pool(name="ph0", bufs=4) as ph0:
                def build(tab, xcol0, nch):
                    for j in range(nch):
                        xt = ph0.tile([P, 512], bf16, tag="xt")
                        nc.sync.dma_start(
                            out=xt[:],
                            in_=xT[:, xcol0 + j * 512:xcol0 + (j + 1) * 512])
                        hp = ps0.tile([P, 512], f32, tag="hp")
                        for s in range(4):
                            nc.tensor.matmul(hp[:, s * P:(s + 1) * P],
                                             lhsT=xt[:, s * P:(s + 1) * P],
                                             rhs=wcT[:], start=True,
                                             stop=True)
                        ts = ph0.tile([P, 512], bf16, tag="ts")
                        nc.scalar.activation(ts[:], hp[:], func=FT.Identity,
                                             scale=1.0)
                        nc.sync.dma_start(
                            out=tab[j * 512:(j + 1) * 512, :].rearrange(
                                "(s p) o -> p s o", p=P),
                            in_=ts[:].rearrange("p (s o) -> p s o", s=4),
                        )

                build(table_hi, 0, hi_rows // 512)
                build(table_lo, hi_rows, lo_rows // 512)

        u1 = big.tile([P, ec], bf16)
        stats = consts.tile([P, nstat, 6], f32)

        def bn_coeffs(g_s, be_s, nck, corr=None):
            se = red.tile([P, nstat], f32, tag="se")
            nc.vector.tensor_mul(se[:, :nck], stats[:, :nck, 0],
                                 stats[:, :nck, 1])
            so = red.tile([P, nstat], f32, tag="so")
            nc.vector.tensor_mul(so[:, :nck], stats[:, :nck, 3],
                                 stats[:, :nck, 4])
            qe = red.tile([P, nstat], f32, tag="qe")
            nc.vector.tensor_mul(qe[:, :nck], se[:, :nck], stats[:, :nck, 1])
            nc.vector.tensor_add(qe[:, :nck], qe[:, :nck], stats[:, :nck, 2])
            qo = red.tile([P, nstat], f32, tag="qo")
            nc.vector.tensor_mul(qo[:, :nck], so[:, :nck], stats[:, :nck, 4])
            nc.vector.tensor_add(qo[:, :nck], qo[:, :nck], stats[:, :nck, 5])
            nc.vector.tensor_add(se[:, :nck], se[:, :nck], so[:, :nck])
            nc.vector.tensor_add(qe[:, :nck], qe[:, :nck], qo[:, :nck])
            sq = red.tile([P, 2], f32, tag="sq")
            nc.vector.tensor_reduce(sq[:, 0:1], se[:, :nck],
                                    axis=mybir.AxisListType.X,
                                    op=mybir.AluOpType.add)
            nc.vector.tensor_reduce(sq[:, 1:2], qe[:, :nck],
                                    axis=mybir.AxisListType.X,
                                    op=mybir.AluOpType.add)
            cc_in = dram.tile([P, 2], f32, tag="cc_in")
            nc.sync.dma_start(out=cc_in[:], in_=sq[:])
            cc_out = dram.tile([P, 2], f32, tag="cc_out")
            nc.gpsimd.collective_compute(
                "AllReduce", mybir.AluOpType.add, replica_groups=grp_all,
                ins=[cc_in[:].opt()], outs=[cc_out[:].opt()])
            sqg = red.tile([P, 2], f32, tag="sqg")
            nc.sync.dma_start(out=sqg[:], in_=cc_out[:])
            if corr is not None:
                v, vq = corr
                t = red.tile([P, 2], f32, tag="tcorr")
                nc.vector.tensor_scalar_mul(t[:, 0:1], v[:], float(n_pad_tot))
                nc.vector.tensor_scalar_mul(t[:, 1:2], vq[:], float(n_pad_tot))
                nc.vector.tensor_sub(sqg[:], sqg[:], t[:])
            mu = red.tile([P, 1], f32, tag="mu")
            nc.vector.tensor_scalar_mul(mu[:], sqg[:, 0:1], 1.0 / e_total)
            var = red.tile([P, 1], f32, tag="var")
            nc.vector.tensor_scalar_mul(var[:], sqg[:, 1:2], 1.0 / e_total)
            mu2 = red.tile([P, 1], f32, tag="mu2")
            nc.vector.tensor_mul(mu2[:], mu[:], mu[:])
            nc.vector.tensor_sub(var[:], var[:], mu2[:])
            a = red.tile([P, 1], f32, tag="a")
            nc.scalar.activation(a[:], var[:], func=FT.Sqrt, bias=eps_s[:],
                                 scale=1.0)
            nc.vector.reciprocal(a[:], a[:])
            nc.vector.tensor_mul(a[:], a[:], g_s[:])
            c = red.tile([P, 1], f32, tag="c")
            nc.vector.tensor_mul(c[:], mu[:], a[:])
            nc.vector.tensor_sub(c[:], be_s[:], c[:])
            return a, c

        with (
            tc.tile_pool(name="psA", bufs=4, space="PSUM") as psA,
            tc.tile_pool(name="psB", bufs=2, space="PSUM") as psB,
            tc.tile_pool(name="psS", bufs=1, space="PSUM") as psS,
            tc.tile_pool(name="ea", bufs=2) as eap,
            tc.tile_pool(name="sr", bufs=3) as srp,
            tc.tile_pool(name="gp", bufs=3) as gp,
            tc.tile_pool(name="wr", bufs=3) as wrp,
            tc.tile_pool(name="sp", bufs=3) as Sp,
            tc.tile_pool(name="op", bufs=2) as op,
        ):
            # ---- pass A ------------------------------------------------
            # dst gathers, one per group (queued up front; gp bufs throttle)
            g_tiles = {}
            for gi, (off, L, dst_hi) in enumerate(lay.groups):
                if gi in idx_pre:
                    di = idx_pre[gi]
                else:
                    di = gidx.tile([P, GROUP // 16], i16, tag="di")
                    nc.sync.dma_start(out=di[:, :L // 16],
                                      in_=didx[:, off // 16:(off + L) // 16])
                gdst = gp.tile([P, GROUP], bf16, tag="gdst")
                base = table_hi if dst_hi else table_lo
                nc.gpsimd.dma_gather(
                    out_ap=gdst[:, :L].rearrange("p (a s) -> p a s", a=1),
                    in_ap=base, idxs_ap=di[:, :L // 16],
                    num_idxs=L, num_idxs_reg=L, elem_size=P,
                    transpose=True, single_packet=False)
                g_tiles[gi] = (gdst, off)

            # group-level ea staging
            ea_tiles = {}

            run_tiles = {}
            caps_hi = lay.caps[0]

            for ci, (off, gi, wins) in enumerate(lay.chunks):
                bkt = 0 if off < caps_hi else 1
                goff, gL, _ = lay.groups[gi]
                if gi not in ea_tiles:
                    et = eap.tile([P, GROUP], bf16, tag="ea")
                    nc.sync.dma_start(out=et[:, :gL],
                                      in_=eaT[:, goff:goff + gL])
                    ea_tiles = {gi: et}
                et = ea_tiles[gi]
                rel = off - goff

                up = psA.tile([P, CHUNK], f32, tag="up")
                nc.tensor.matmul(up[:], lhsT=w1bT[:], rhs=et[:, rel:rel + CHUNK],
                                 start=True, stop=(len(wins) == 0),
                                 skip_group_check=bool(wins))

                if wins:
                    clo = min(w[3] for w in wins)
                    chi = max(w[4] for w in wins)
                    st = srp.tile([1, CHUNK], fp16, tag="sr")
                    nc.scalar.dma_start(out=st[:, clo:chi],
                                        in_=srel[:, off + clo:off + chi])
                    bps = psB.tile([P, CHUNK], f32, tag="bps")
                    nc.tensor.matmul(bps[:, clo:chi], lhsT=ones16[:],
                                     rhs=st[:, clo:chi],
                                     start=True, stop=True)
                    for wi, (run_i, k, dw, a, b) in enumerate(wins):
                        rkey = (bkt, run_i)
                        if rkey not in run_tiles:
                            wt = wrp.tile([P, RUN], bf16, tag="wt")
                            r0 = lay.runs[run_i]
                            rt = table_hi if r0 < hi_rows else table_lo
                            rr = r0 if r0 < hi_rows else r0 - hi_rows
                            nc.scalar.dma_start(
                                out=wt[:].rearrange("p (k f) -> p k f",
                                                    k=RUN // P),
                                in_=rt[rr:rr + RUN, :].rearrange(
                                    "(k p) f -> p k f", p=P))
                            run_tiles[rkey] = wt
                        wt = run_tiles[rkey]
                        S_w = Sp.tile([P, CHUNK], bf16, tag="S")
                        nc.vector.tensor_scalar(
                            out=S_w[:, a:b], in0=bps[:, a:b],
                            scalar1=iota_s[:, dw:dw + 1], scalar2=None,
                            op0=mybir.AluOpType.is_equal)
                        nc.tensor.matmul(up[:, a:b],
                                         lhsT=wt[:, k * P:(k + 1) * P],
                                         rhs=S_w[:, a:b], start=False,
                                         stop=(wi == len(wins) - 1),
                                         skip_group_check=True)

                gdst, g_off = g_tiles[gi]
                grel = off - g_off
                nc.vector.tensor_add(u1[:, off:off + CHUNK], up[:],
                                     gdst[:, grel:grel + CHUNK])
                nc.vector.bn_stats(stats[:, ci, :], u1[:, off:off + CHUNK])

            a1, c1 = bn_coeffs(g1_s, be1_s, nchunk)

            # pad columns have u1 == 0 -> u2_pad = W2 @ relu(c1), constant
            rc = red.tile([P, 1], f32, tag="rc")
            nc.scalar.activation(rc[:], c1[:], func=FT.Relu)
            rcb = red.tile([P, 1], bf16, tag="rcb")
            nc.vector.tensor_copy(rcb[:], rc[:])
            vp = psS.tile([P, 1], f32, tag="vp")
            nc.tensor.matmul(vp[:], lhsT=w2T[:], rhs=rcb[:],
                             start=True, stop=True)
            v2 = red.tile([P, 1], f32, tag="v2")
            nc.vector.tensor_copy(v2[:], vp[:])
            v2q = red.tile([P, 1], f32, tag="v2q")
            nc.vector.tensor_mul(v2q[:], v2[:], v2[:])

            # ---- pass B: z1 = relu(a1*u1+c1) in place; stats of W2@z1 ---
            for k in range(nchunk):
                off = k * CHUNK
                nc.scalar.activation(u1[:, off:off + CHUNK],
                                     u1[:, off:off + CHUNK],
                                     func=FT.Relu, scale=a1[:], bias=c1[:])
                up = psA.tile([P, CHUNK], f32, tag="up")
                nc.tensor.matmul(up[:], lhsT=w2T[:],
                                 rhs=u1[:, off:off + CHUNK],
                                 start=True, stop=True)
                nc.vector.bn_stats(stats[:, k, :], up[:])

            a2, c2 = bn_coeffs(g2_s, be2_s, nchunk, corr=(v2, v2q))

            # ---- pass C: out = relu(a2*(W2@z1)+c2), staged per group ----
            for base in range(0, ec, GROUP):
                gL = min(GROUP, ec - base)
                ot = op.tile([P, GROUP], bf16, tag="ot")
                for off in range(base, base + gL, CHUNK):
                    up = psA.tile([P, CHUNK], f32, tag="up")
                    nc.tensor.matmul(up[:], lhsT=w2T[:],
                                     rhs=u1[:, off:off + CHUNK],
                                     start=True, stop=True)
                    r = off - base
                    nc.scalar.activation(ot[:, r:r + CHUNK], up[:],
                                         func=FT.Relu, scale=a2[:],
                                         bias=c2[:])
                nc.sync.dma_start(out=outT[:, base:base + gL],
                                  in_=ot[:, :gL])

    nc.compile()
    return nc


def _wrap16(a):
    """linear [L] -> [16, L/16] wrapped, tiled to [128, L/16]."""
    w = np.ascontiguousarray(a.reshape(-1, 16).T)
    return np.tile(w, (8, 1))


def host_prep(x, edge_index, edge_attr, n_cores):
    """Shard, bucket by dst-region, sort by src, build layout + per-core
    arrays."""
    n = x.shape[0]
    e = edge_attr.shape[0]
    ec0 = e // n_cores
    nhi, hi_rows, lo_rows, npad = table_layout(n)

    src_all = edge_index[0].astype(np.int64)
    dst_all = edge_index[1].astype(np.int64)

    per_core = []
    counts = np.zeros((n_cores, 2), np.int64)
    for c in range(n_cores):
        sl = slice(c * ec0, (c + 1) * ec0)
        s, d = src_all[sl], dst_all[sl]
        hi = (d >= SPLIT).astype(np.int64)
        order = np.argsort(hi * (1 << 32) + s, kind="stable")
        counts[c, 1] = int(hi.sum())          # bucket 1 = hi
        counts[c, 0] = ec0 - counts[c, 1]
        per_core.append((s, d, hi, order))

    # bucket order: hi first (table hi region builds first), caps %512
    caps = (int(_r512(counts[:, 1].max())), int(_r512(counts[:, 0].max())))
    ec = caps[0] + caps[1]
    bucket_off = {1: 0, 0: caps[0]}

    zero_hi = nhi                 # local idx of zero row in hi region
    zero_lo = SPLIT               # local idx of zero row in lo region

    # groups (shared across cores: same caps)
    groups = []
    for b, cap in ((1, caps[0]), (0, caps[1])):
        off = bucket_off[b]
        rem = cap
        while rem > 0:
            L = min(GROUP, rem)
            groups.append((off, L, b == 1))
            off += L
            rem -= L

    # per-core padded arrays + union chunk windows
    all_srel = []
    all_didx = []
    all_eacols = []
    all_inv = []
    win_ranges = [dict() for _ in range(ec // CHUNK)]
    w0_arr = np.zeros(ec // CHUNK, np.int64)

    # first pass: compute padded src arrays to derive union windows
    src_p_all = []
    for c in range(n_cores):
        s, d, hi, order = per_core[c]
        src_p = np.full(ec, -1, np.int64)
        dst_p = np.empty(ec, np.int64)
        ea_cols = np.full(ec, -1, np.int64)
        # order sorts by (hi, src): lo bucket first in order, but hi bucket
        # comes first in columns.
        n_lo = int(counts[c, 0])
        idx_lo = order[:n_lo]
        idx_hi = order[n_lo:]
        for b, idx_b in ((1, idx_hi), (0, idx_lo)):
            cnt = len(idx_b)
            off = bucket_off[b]
            pos = off + np.arange(cnt)
            src_p[pos] = s[idx_b]
            dst_p[pos] = d[idx_b] - (SPLIT if b == 1 else 0)
            ea_cols[pos] = idx_b
            padr = np.arange(off + cnt, off + (caps[0] if b == 1 else caps[1]))
            dst_p[padr] = zero_hi if b == 1 else zero_lo
        inv = np.empty(ec0, np.int64)
        inv[idx_hi] = bucket_off[1] + np.arange(len(idx_hi))
        inv[idx_lo] = bucket_off[0] + np.arange(len(idx_lo))
        src_p_all.append(src_p)
        all_didx.append(dst_p)
        all_eacols.append(ea_cols)
        all_inv.append(inv)

        for ci in range(ec // CHUNK):
            seg = src_p[ci * CHUNK:(ci + 1) * CHUNK]
            for w in np.unique(seg[seg >= 0] // P):
                pos = np.nonzero(seg // P == w)[0]
                a, b = int(pos[0]), int(pos[-1]) + 1
                cur = win_ranges[ci].get(int(w))
                if cur is None:
                    win_ranges[ci][int(w)] = [a, b]
                else:
                    cur[0] = min(cur[0], a)
                    cur[1] = max(cur[1], b)

    # per-chunk base window + srel arrays (shared w0 across cores)
    chunks = []
    run_index = {}
    runs = []
    gi_of_off = {off: i for i, (off, L, _) in enumerate(groups)
                 for off in range(off, off + L, CHUNK)}
    for ci in range(ec // CHUNK):
        wins = sorted(win_ranges[ci])
        off = ci * CHUNK
        wl = []
        if wins:
            w0 = wins[0]
            w0_arr[ci] = w0
            assert wins[-1] - w0 < MAXW, f"chunk {ci} spans {wins}"
            for w in wins:
                node0 = w * P
                if node0 >= SPLIT:
                    row0 = node0 - SPLIT          # hi region
                else:
                    row0 = hi_rows + node0        # lo region
                run0 = (row0 // RUN) * RUN
                if run0 not in run_index:
                    run_index[run0] = len(runs)
                    runs.append(run0)
                a, b = win_ranges[ci][w]
                wl.append((run_index[run0], (row0 - run0) // P, w - w0,
                           a, b))
        chunks.append((off, gi_of_off[off], wl))

    for c in range(n_cores):
        src_p = src_p_all[c]
        srel = np.full(ec, -1.0, np.float32)
        for ci in range(ec // CHUNK):
            seg = src_p[ci * CHUNK:(ci + 1) * CHUNK]
            m = seg >= 0
            srel[ci * CHUNK:(ci + 1) * CHUNK][m] = seg[m] - w0_arr[ci] * P
        assert srel.max() < 2048
        all_srel.append(srel.astype(FP16))

    lay = Layout(caps, chunks, groups, runs, n, e)
    return lay, all_srel, all_didx, all_eacols, all_inv


def make_in_maps(x, edge_index, edge_attr, W_lin, b_lin, W1, g1, be1, W2,
                 g2, be2, n_cores):
    n = x.shape[0]
    nhi, hi_rows, lo_rows, npad = table_layout(n)
    lay, all_srel, all_didx, all_eacols, all_inv = host_prep(
        x, edge_index, edge_attr, n_cores)
    ec = lay.ec
    ec0 = edge_attr.shape[0] // n_cores

    xbf = x.astype(BF16)
    xT = np.zeros((P, npad), dtype=BF16)
    xT[:, 0:nhi] = xbf[SPLIT:n].T
    xT[:, hi_rows:hi_rows + SPLIT] = xbf[0:SPLIT].T

    iota = (np.arange(P)[:, None]
            + P * np.arange(MAXW)[None, :]).astype(np.float32)

    f32c = np.ascontiguousarray
    wlin_h = f32c(W_lin.astype(np.float32))
    w1_h = f32c(W1.astype(np.float32))
    w2_h = f32c(W2.astype(np.float32))
    blin_h = f32c(b_lin.astype(np.float32).reshape(P, 1))
    g1_h = f32c(g1.astype(np.float32).reshape(P, 1))
    be1_h = f32c(be1.astype(np.float32).reshape(P, 1))
    g2_h = f32c(g2.astype(np.float32).reshape(P, 1))
    be2_h = f32c(be2.astype(np.float32).reshape(P, 1))

    eabf = edge_attr.astype(BF16)

    in_maps = []
    for c in range(n_cores):
        ea_cols = all_eacols[c]
        eaT = np.zeros((P, ec), dtype=BF16)
        real = ea_cols >= 0
        eaT[:, real] = eabf[c * ec0 + ea_cols[real]].T
        dw = np.zeros((P, ec // 16), np.int16)
        for off, L, _ in lay.groups:
            dw[:, off // 16:(off + L) // 16] = _wrap16(
                all_didx[c][off:off + L].astype(np.int16))
        in_maps.append({
            "eaT": eaT, "xT": xT, "srel": all_srel[c].reshape(1, ec),
            "didx": dw, "iota": iota,
            "wlin": wlin_h, "w1": w1_h, "w2": w2_h, "blin": blin_h,
            "g1": g1_h, "be1": be1_h, "g2": g2_h, "be2": be2_h,
        })
    return lay, in_maps, all_inv


_GRAPH_CACHE = {}


def get_graph(lay: Layout):
    if lay.key not in _GRAPH_CACHE:
        _GRAPH_CACHE[lay.key] = build_graph(lay)
    return _GRAPH_CACHE[lay.key]


def kernel(x, edge_index, edge_attr, W_lin, b_lin, W1, b1, g1, be1, W2, b2,
           g2, be2):
    """Full-input entry point: shard, run on 8 NeuronCores, gather."""
    x = np.asarray(x)
    edge_index = np.asarray(edge_index)
    edge_attr = np.asarray(edge_attr)
    e = edge_attr.shape[0]
    ec0 = e // N_CORES

    lay, in_maps, invs = make_in_maps(
        x, edge_index, edge_attr, np.asarray(W_lin), np.asarray(b_lin),
        np.asarray(W1), np.asarray(g1), np.asarray(be1), np.asarray(W2),
        np.asarray(g2), np.asarray(be2), N_CORES)
    nc = get_graph(lay)
    res = run_bass_kernel_spmd(nc, in_maps, core_ids=list(range(N_CORES)))
    out = np.empty((e, NIN), dtype=np.float32)
    for c in range(N_CORES):
        oT = np.asarray(res.results[c]["outT"], dtype=np.float32)
        out[c * ec0:(c + 1) * ec0] = oT.T[invs[c]]
    return out
